# revision 7
# baseline (speedup 1.0000x reference)
"""BaselineGNN (SAGEConv-mean x3 + BN + relu, graph mean-pool, MLP head) on 8 Trainium2 cores.

Strategy (v2):
  - Nodes/edges sharded by graph across 8 cores; each core owns the destination
    nodes (and all in-edges) of 512 consecutive graphs.
  - Node features in a replicated [8*PN, 128] bf16 DRAM table; per-edge source
    rows fetched with dma_gather. Edges are bucketed by (supergroup of 16
    dst-windows, src-window, dst-window) so one gather call covers a whole
    (supergroup, src-window) span (~8k rows) -> few, large SWDGE calls.
  - Aggregation: per 128-edge tile, S = one-hot(dst slot) built on DVE in
    multi-tile is_equal ops; PE accumulates g.T @ S into a [128, 2048] f32
    PSUM supergroup tile (dst windows of 128).
  - Raw sums are scaled by 1/deg via a prebuilt column-broadcast invdeg matrix
    during the PSUM->SBUF copy (one DVE op per supergroup) - no per-tile scaling.
  - x_new_T = Wl.T@agg_T + Wr.T@x_T per 512-node block, interleaved per
    supergroup; BN batch stats via ScalarE accumulators + [128,2] AllReduce;
    scale+shift+relu fused in one ScalarE activation.
  - Updated shard PE-transposed back to [node, dim] rows, AllGathered into the
    next layer's table. Graph mean-pool via fused (is_equal * inv_cnt) one-hot
    matmuls; 2-layer MLP head.
"""
import os
import numpy as np
import ml_dtypes

from concourse import bass, bacc, mybir
from concourse.bass_utils import run_bass_kernel_spmd
from concourse.masks import make_identity
import concourse.tile as tile

BF16 = mybir.dt.bfloat16
F16 = mybir.dt.float16
F32 = mybir.dt.float32
I16 = mybir.dt.int16
I32 = mybir.dt.int32

C = 8            # cores
D = 128          # feature dim
HD = 64          # head hidden dim
L = 3            # layers
WDST = 128       # dst window (one-hot width / PSUM sub-window)
KSG = 16         # dst windows per supergroup (PSUM tile = [128, KSG*128] f32)
BLK = 512        # node block for update matmuls
TS = 16          # matmul tiles per S-build op
MAXCALL = 2048   # max indices per dma_gather call (SWDGE ring = 128 descs x 16 idxs)
BN_EPS = 1e-5

LAST_RESULT = None


def _ceil(a, b):
    return -(-a // b) * b


class Plan:
    pass


def _preprocess(x, esrc, edst, bids):
    p = Plan()
    N = x.shape[0]
    G = 4096 if N > 5000 else int(bids.max()) + 1
    GPC = G // C
    p.N, p.G, p.GPC = N, G, GPC

    node_start = np.searchsorted(bids, np.arange(0, G + 1, GPC)).astype(np.int64)
    n_c = np.diff(node_start)
    PN = int(_ceil(int(n_c.max()), BLK))
    p.PN = PN
    p.NB = PN // 128
    p.NBLK = PN // BLK
    NW = PN // WDST
    p.NW = NW
    NSG = -(-NW // KSG)
    p.NSG = NSG
    WS = 2 * PN
    assert WS <= 32767, f"src window {WS} exceeds int16"
    NSW = -(-C * PN // WS)
    p.WS, p.NSW = WS, NSW

    own = np.repeat(np.arange(C), n_c)
    local = np.arange(N) - node_start[own]
    row = own * PN + local

    deg = np.bincount(edst, minlength=N).astype(np.float32)
    invdeg = (1.0 / np.maximum(deg, 1.0)).astype(np.float32)

    e_own = own[edst]
    e_dl = local[edst]
    e_sr = row[esrc]
    e_sw = e_sr // WS
    e_sl = (e_sr % WS).astype(np.int16)
    e_w = e_dl // WDST
    e_sg = e_w // KSG
    # bucket: (supergroup | srcwin | dstwin-in-sg)
    NBUCK = NSG * NSW * KSG
    key = e_own * NBUCK + (e_sg * NSW + e_sw) * KSG + (e_w % KSG)
    order = np.lexsort((e_sr, key))

    counts = np.bincount(key, minlength=C * NBUCK).reshape(C, NBUCK)
    maxc = counts.max(axis=0)
    padded = (_ceil(maxc.astype(np.int64), 128)).astype(np.int64)
    # zero out buckets of nonexistent windows in the (ragged) last supergroup
    for sg in range(NSG):
        kw = min(KSG, NW - sg * KSG)
        for s in range(NSW):
            for wl in range(kw, KSG):
                padded[(sg * NSW + s) * KSG + wl] = 0
    # every PSUM bank (4 windows) needs >=1 matmul so it gets zeroed+written
    for wb in range(NW // 4):
        bids_w = [((w // KSG) * NSW + s) * KSG + (w % KSG)
                  for w in range(wb * 4, wb * 4 + 4) for s in range(NSW)]
        if padded[bids_w].sum() == 0:
            padded[bids_w[0]] = 128
    boff = np.concatenate([[0], np.cumsum(padded)])
    EP = int(boff[-1])
    p.EP = EP

    # gather calls: per (sg, srcwin), chopped at MAXCALL (128-aligned)
    p.calls = []          # [sg] -> list of (sw, pos, n)
    p.sg_pos = []         # [sg] -> (start, end) edge-offset range
    p.tiles_sg = []       # [sg] -> list of (col, w_local)
    p.wfirst, p.wlast = [], []
    for sg in range(NSG):
        kw = min(KSG, NW - sg * KSG)
        calls = []
        sg_start = int(boff[(sg * NSW + 0) * KSG])
        sg_end = int(boff[min((sg * NSW + NSW - 1) * KSG + KSG, NBUCK)])
        for s in range(NSW):
            b0 = (sg * NSW + s) * KSG
            pos = int(boff[b0])
            m = int(padded[b0:b0 + KSG].sum())
            if m > 0:
                nchunks = -(-m // MAXCALL)
                csz = _ceil(-(-m // nchunks), 128)
                while m > 0:
                    n = min(csz, m)
                    calls.append((s, pos, n))
                    pos += n
                    m -= n
        p.calls.append(calls)
        p.sg_pos.append((sg_start, sg_end))
        tl = []
        for s in range(NSW):
            for wl in range(kw):
                b = (sg * NSW + s) * KSG + wl
                for t in range(int(padded[b]) // 128):
                    tl.append((int(boff[b]) // 128 + t, wl))
        p.tiles_sg.append(tl)
        wf = {}
        wl_ = {}
        for (col, w) in tl:
            wb = w // 4
            if wb not in wf:
                wf[wb] = col
            wl_[wb] = col
        p.wfirst.append(wf)
        p.wlast.append(wl_)
    p.max_sg_cols = max(e - s for s, e in p.sg_pos)

    # per-core edge arrays in padded layout
    key_sorted = key[order]
    core_edges = np.searchsorted(key_sorted, np.arange(0, C * NBUCK + 1, NBUCK))
    p.eidx, p.ed = [], []
    for c in range(C):
        sel = order[core_edges[c]:core_edges[c + 1]]
        k_loc = key[sel] - c * NBUCK
        bstart = np.searchsorted(k_loc, np.arange(NBUCK))
        r = np.arange(len(sel)) - bstart[k_loc]
        pos = boff[k_loc] + r
        idx_arr = np.zeros(EP, np.int16)
        d_arr = np.full(EP, -1.0, np.float32)
        idx_arr[pos] = e_sl[sel]
        d_arr[pos] = (e_dl[sel] % WDST).astype(np.float32)
        eidx16 = idx_arr.reshape(EP // 16, 16).T.copy()
        p.eidx.append(np.tile(eidx16, (8, 1)))
        p.ed.append(d_arr.reshape(EP // 128, 128).T.astype(ml_dtypes.bfloat16))

    # initial table + per-core node-side arrays
    tbl0 = np.zeros((C * PN, D), ml_dtypes.bfloat16)
    tbl0[row] = x.astype(ml_dtypes.bfloat16)
    p.table0 = tbl0
    p.xt0, p.invdegB = [], []
    p.wpool, p.bloc, p.mask_tail = [], [], []
    cnt = np.bincount(bids, minlength=G).astype(np.float32)
    inv_cnt = (1.0 / np.maximum(cnt, 1.0)).astype(np.float32)
    MT = min(PN, 1024)
    p.MT = MT
    for c in range(C):
        nc_ = int(n_c[c])
        xt = np.zeros((D, PN), ml_dtypes.bfloat16)
        xt[:, :nc_] = x[node_start[c]:node_start[c + 1]].T.astype(ml_dtypes.bfloat16)
        p.xt0.append(xt)
        iv = np.zeros(PN, np.float32)
        iv[:nc_] = invdeg[node_start[c]:node_start[c + 1]]
        p.invdegB.append(np.tile(iv[None, :].astype(ml_dtypes.bfloat16), (128, 1)))
        wp = np.zeros(PN, np.float32)
        bl = np.full(PN, -1.0, np.float32)
        gids = bids[node_start[c]:node_start[c + 1]]
        wp[:nc_] = inv_cnt[gids]
        bl[:nc_] = (gids - c * GPC).astype(np.float32)
        p.wpool.append(wp.reshape(PN // 128, 128).T.copy())
        p.bloc.append(bl.reshape(PN // 128, 128).T.copy())
        mt = np.zeros(MT, ml_dtypes.bfloat16)
        valid_in_tail = nc_ - (PN - MT)
        if valid_in_tail > 0:
            mt[:valid_in_tail] = 1.0
        p.mask_tail.append(np.tile(mt[None, :], (128, 1)))
    return p


def _build(p, Wl, Wr, gamma, beta, hW1, hb1, hW2, hb2):
    PN, NW, NB, NBLK, NSW, WS, EP = p.PN, p.NW, p.NB, p.NBLK, p.NSW, p.WS, p.EP
    NSG, GPC = p.NSG, p.GPC
    nc = bacc.Bacc('TRN2', target_bir_lowering=False, debug=False,
                   num_devices=C, num_swdge_queues=4, dynamic_dma_scratch_size=32768)

    # ---- parameters ----
    table0 = nc.declare_dram_parameter("table0", [C * PN, D], BF16, isOutput=False)
    xt0 = nc.declare_dram_parameter("xt0", [D, PN], BF16, isOutput=False)
    eidx = nc.declare_dram_parameter("eidx", [128, EP // 16], I16, isOutput=False)
    ed = nc.declare_dram_parameter("ed", [128, EP // 128], BF16, isOutput=False)
    ivb_p = nc.declare_dram_parameter("invdegB", [128, PN], BF16, isOutput=False)
    wl_p = nc.declare_dram_parameter("wl", [L, D, D], BF16, isOutput=False)
    wr_p = nc.declare_dram_parameter("wr", [L, D, D], BF16, isOutput=False)
    gb_p = nc.declare_dram_parameter("gb", [D, L, 2], F32, isOutput=False)
    wpool_p = nc.declare_dram_parameter("wpool", [128, NB], F32, isOutput=False)
    bloc_p = nc.declare_dram_parameter("bloc", [128, NB], F32, isOutput=False)
    mtail_p = nc.declare_dram_parameter("mtail", [128, p.MT], BF16, isOutput=False)
    w1_p = nc.declare_dram_parameter("w1", [D, HD], BF16, isOutput=False)
    b1_p = nc.declare_dram_parameter("b1", [HD, 1], F32, isOutput=False)
    w2_p = nc.declare_dram_parameter("w2", [HD, 1], BF16, isOutput=False)
    b2_p = nc.declare_dram_parameter("b2", [1, 1], F32, isOutput=False)
    out_p = nc.declare_dram_parameter("out", [GPC], F32, isOutput=True)

    # ---- internal DRAM ----
    tables = [table0]
    shards = []
    for l in range(1, L):
        tables.append(nc.dram_tensor(f"table{l}", [C * PN, D], BF16, addr_space="Shared"))
        shards.append(nc.dram_tensor(f"shard{l}", [PN, D], BF16))
    bnin = [nc.dram_tensor(f"bnin{l}", [D, 2], F32) for l in range(L)]
    bnout = [nc.dram_tensor(f"bnout{l}", [D, 2], F32, addr_space="Shared") for l in range(L)]
    rg = [list(range(C))]

    from contextlib import ExitStack
    with tile.TileContext(nc) as tc, ExitStack() as es:
        const = es.enter_context(tc.tile_pool(name="const", bufs=1))
        big = es.enter_context(tc.tile_pool(name="big", bufs=1))
        eidxp = es.enter_context(tc.tile_pool(name="eidxp", bufs=3))
        featp = es.enter_context(tc.tile_pool(name="feat", bufs=6))
        sp = es.enter_context(tc.tile_pool(name="sel", bufs=8))
        aggsb = es.enter_context(tc.tile_pool(name="aggsb", bufs=2))
        sqp = es.enter_context(tc.tile_pool(name="sqp", bufs=2))
        headp = es.enter_context(tc.tile_pool(name="headp", bufs=1))
        smallp = es.enter_context(tc.tile_pool(name="small", bufs=4))
        aggps = es.enter_context(tc.tile_pool(name="aggps", bufs=1, space="PSUM"))
        zps = es.enter_context(tc.tile_pool(name="zps", bufs=2, space="PSUM"))
        tps = es.enter_context(tc.tile_pool(name="tps", bufs=2, space="PSUM"))
        tbufp = es.enter_context(tc.tile_pool(name="tbuf", bufs=4))

        # ---- persistent constants ----
        iota_i = const.tile([128, WDST], I32)
        nc.gpsimd.iota(iota_i[:], pattern=[[1, WDST]], base=0, channel_multiplier=0)
        iota128 = const.tile([128, WDST], BF16)
        nc.vector.tensor_copy(out=iota128[:], in_=iota_i[:])
        iotaG_i = const.tile([128, GPC], I32)
        nc.gpsimd.iota(iotaG_i[:], pattern=[[1, GPC]], base=0, channel_multiplier=0)
        iotaG = const.tile([128, GPC], F16)
        nc.vector.tensor_copy(out=iotaG[:], in_=iotaG_i[:])
        ident = const.tile([128, 128], BF16)
        make_identity(nc, ident[:])

        wl_s = const.tile([128, L * D], BF16)
        wr_s = const.tile([128, L * D], BF16)
        for l in range(L):
            nc.sync.dma_start(out=wl_s[:, l * D:(l + 1) * D], in_=wl_p[l])
            nc.sync.dma_start(out=wr_s[:, l * D:(l + 1) * D], in_=wr_p[l])
        gb_s = const.tile([128, L, 2], F32)
        nc.sync.dma_start(out=gb_s[:], in_=gb_p[:])
        w1_s = const.tile([D, HD], BF16)
        nc.sync.dma_start(out=w1_s[:], in_=w1_p[:])
        b1_s = const.tile([HD, 1], F32)
        nc.sync.dma_start(out=b1_s[:], in_=b1_p[:])
        w2_s = const.tile([HD, 1], BF16)
        nc.sync.dma_start(out=w2_s[:], in_=w2_p[:])
        b2_s = const.tile([1, 1], F32)
        nc.sync.dma_start(out=b2_s[:], in_=b2_p[:])
        wpool_s = const.tile([128, NB], F32)
        nc.sync.dma_start(out=wpool_s[:], in_=wpool_p[:])
        bloc_s = const.tile([128, NB], F32)
        nc.sync.dma_start(out=bloc_s[:], in_=bloc_p[:])
        mtail_s = const.tile([128, p.MT], BF16)
        nc.sync.dma_start(out=mtail_s[:], in_=mtail_p[:])
        eps_s = const.tile([128, 1], F32)
        nc.vector.memset(eps_s[:], BN_EPS)

        ed_s = big.tile([128, EP // 128], BF16, tag="ed")
        nc.sync.dma_start(out=ed_s[:], in_=ed[:])
        ivb_s = big.tile([128, PN], BF16, tag="ivb")
        nc.sync.dma_start(out=ivb_s[:], in_=ivb_p[:])

        xt = [big.tile([D, PN], BF16, tag="xt0", name="xt_a"),
              big.tile([D, PN], BF16, tag="xt1", name="xt_b")]
        nc.sync.dma_start(out=xt[0][:], in_=xt0[:])
        sq_scr = sqp.tile([128, BLK], F32, tag="sqscr")

        MAXSGC = _ceil(p.max_sg_cols // 16, 64)

        scope = nc.named_scope
        for l in range(L):
            tbl = tables[l]
            xt_cur = xt[l % 2]
            xt_nxt = xt[(l + 1) % 2]

            es_l = ExitStack(); es_l.enter_context(scope(f"agg{l}"))
            parts = smallp.tile([128, 2, NBLK], F32, tag="parts", name=f"parts{l}")
            qrot = 0
            for sg in range(NSG):
                kw = min(KSG, NW - sg * KSG)
                cols = kw * 128
                sg_start, sg_end = p.sg_pos[sg]
                eidx_sg = eidxp.tile([128, MAXSGC], I16, tag="eidx", name=f"eidx{l}_{sg}")
                nc.sync.dma_start(out=eidx_sg[:, :(sg_end - sg_start) // 16],
                                  in_=eidx.ap()[:, sg_start // 16:sg_end // 16])
                agg_ps = aggps.tile([128, cols], F32, tag="aggps", name=f"aggps{l}_{sg}")
                gbufs = []
                for (sw, pos, n) in p.calls[sg]:
                    g = featp.tile([128, n // 128, D], BF16, tag="g", name=f"g{l}_{sg}_{len(gbufs)}")
                    nc.gpsimd.dma_gather(
                        out_ap=g[:],
                        in_ap=tbl.ap()[sw * WS:(sw + 1) * WS],
                        idxs_ap=eidx_sg[:, (pos - sg_start) // 16:(pos - sg_start + n) // 16],
                        num_idxs=n, num_idxs_reg=n, elem_size=D,
                        single_packet=(n <= 1024),
                        queue_num=qrot % 4,
                    )
                    qrot += 1
                    gbufs.append((pos // 128, n // 128, g))
                tl = p.tiles_sg[sg]
                wf, wla = p.wfirst[sg], p.wlast[sg]
                gi = 0
                for ci in range(0, len(tl), TS):
                    chunk = tl[ci:ci + TS]
                    ts = len(chunk)
                    c0 = chunk[0][0]
                    S = sp.tile([128, ts, WDST], BF16, tag="S", name=f"S{l}_{sg}_{ci}")
                    nc.vector.tensor_tensor(
                        out=S[:],
                        in0=ed_s[:, c0:c0 + ts].unsqueeze(-1).to_broadcast([128, ts, WDST]),
                        in1=iota128[:].unsqueeze(1).to_broadcast([128, ts, WDST]),
                        op=mybir.AluOpType.is_equal)
                    for j, (col, w) in enumerate(chunk):
                        while not (gbufs[gi][0] <= col < gbufs[gi][0] + gbufs[gi][1]):
                            gi += 1
                        g0, _, g = gbufs[gi]
                        wb = w // 4
                        nc.tensor.matmul(out=agg_ps[:, w * WDST:(w + 1) * WDST],
                                         lhsT=g[:, col - g0, :], rhs=S[:, j, :],
                                         start=(col == wf[wb]), stop=(col == wla[wb]))
                agg_sb = aggsb.tile([128, cols], BF16, tag="aggsb", name=f"aggsb{l}_{sg}")
                nc.vector.tensor_tensor(out=agg_sb[:], in0=agg_ps[:],
                                        in1=ivb_s[:, sg * KSG * 128:sg * KSG * 128 + cols],
                                        op=mybir.AluOpType.mult)
                # ---- update matmuls + BN stat accum for this supergroup ----
                nblk_sg = cols // BLK
                for bi in range(nblk_sg):
                    b = (sg * KSG * 128) // BLK + bi
                    off = bi * BLK
                    g0 = b * BLK
                    z_ps = zps.tile([128, BLK], F32, tag="z", name=f"z{l}_{b}")
                    nc.tensor.matmul(out=z_ps[:], lhsT=wl_s[:, l * D:(l + 1) * D],
                                     rhs=agg_sb[:, off:off + BLK], start=True, stop=False)
                    nc.tensor.matmul(out=z_ps[:], lhsT=wr_s[:, l * D:(l + 1) * D],
                                     rhs=xt_cur[:, g0:g0 + BLK], start=False, stop=True)
                    nc.scalar.activation(out=xt_nxt[:, g0:g0 + BLK], in_=z_ps[:],
                                         func=mybir.ActivationFunctionType.Copy,
                                         accum_out=parts[:, 0, b:b + 1])
                    nc.scalar.activation(out=sq_scr[:], in_=z_ps[:],
                                         func=mybir.ActivationFunctionType.Square,
                                         accum_out=parts[:, 1, b:b + 1])

            es_l.close()
            es_l = ExitStack(); es_l.enter_context(scope(f"bnred{l}"))
            st_loc = smallp.tile([128, 2], F32, tag="stloc", name=f"stloc{l}")
            nc.vector.tensor_reduce(out=st_loc[:], in_=parts[:],
                                    axis=mybir.AxisListType.X, op=mybir.AluOpType.add)
            nc.sync.dma_start(out=bnin[l][:], in_=st_loc[:])
            nc.gpsimd.collective_compute(
                "AllReduce", mybir.AluOpType.add, replica_groups=rg,
                ins=[bnin[l][:]], outs=[bnout[l][:]])
            st = smallp.tile([128, 2], F32, tag="st", name=f"st{l}")
            nc.sync.dma_start(out=st[:], in_=bnout[l][:])

            # scale = gamma * rsqrt(var+eps); shift = beta - mean*scale
            stat = smallp.tile([128, 6], F32, tag="stat", name=f"stat{l}")
            inv_n = 1.0 / float(p.N)
            nc.vector.tensor_scalar(out=stat[:, 0:2], in0=st[:, 0:2], scalar1=inv_n,
                                    scalar2=None, op0=mybir.AluOpType.mult)  # mean, E[x^2]
            nc.vector.tensor_tensor(out=stat[:, 2:3], in0=stat[:, 0:1], in1=stat[:, 0:1],
                                    op=mybir.AluOpType.mult)  # mean^2
            nc.vector.tensor_tensor(out=stat[:, 2:3], in0=stat[:, 1:2], in1=stat[:, 2:3],
                                    op=mybir.AluOpType.subtract)  # var
            nc.scalar.activation(out=stat[:, 3:4], in_=stat[:, 2:3],
                                 func=mybir.ActivationFunctionType.Sqrt, bias=eps_s[:, 0:1])
            nc.vector.reciprocal(out=stat[:, 4:5], in_=stat[:, 3:4])  # rsqrt(var+eps)
            nc.vector.tensor_tensor(out=stat[:, 4:5], in0=stat[:, 4:5],
                                    in1=gb_s[:, l, 0:1], op=mybir.AluOpType.mult)  # scale
            nc.vector.tensor_tensor(out=stat[:, 5:6], in0=stat[:, 0:1], in1=stat[:, 4:5],
                                    op=mybir.AluOpType.mult)
            nc.vector.tensor_tensor(out=stat[:, 5:6], in0=gb_s[:, l, 1:2], in1=stat[:, 5:6],
                                    op=mybir.AluOpType.subtract)  # shift

            es_l.close()
            es_l = ExitStack(); es_l.enter_context(scope(f"bnapp{l}"))
            # ---- BN apply + relu (+ tail mask) ----
            for b in range(NBLK):
                sl = slice(b * BLK, (b + 1) * BLK)
                nc.scalar.activation(out=xt_nxt[:, sl], in_=xt_nxt[:, sl],
                                     func=mybir.ActivationFunctionType.Relu,
                                     scale=stat[:, 4:5], bias=stat[:, 5:6])
            mt0 = PN - p.MT
            nc.vector.tensor_tensor(out=xt_nxt[:, mt0:PN], in0=xt_nxt[:, mt0:PN],
                                    in1=mtail_s[:], op=mybir.AluOpType.mult)

            es_l.close()
            # ---- transpose to [node, dim] + AllGather ----
            if l < L - 1:
                es_l = ExitStack(); es_l.enter_context(scope(f"trans{l}"))
                shard_v = shards[l].ap().rearrange("(k p) d -> p k d", p=128)
                for k in range(NB):
                    t_ps = tps.tile([128, 128], BF16, tag="tps", name=f"tp{l}_{k}")
                    nc.tensor.transpose(out=t_ps[:], in_=xt_nxt[:, k * 128:(k + 1) * 128],
                                        identity=ident[:])
                    t_sb = tbufp.tile([128, 128], BF16, tag="tsb", name=f"ts{l}_{k}")
                    nc.vector.tensor_copy(out=t_sb[:], in_=t_ps[:])
                    nc.sync.dma_start(out=shard_v[:, k, :], in_=t_sb[:])
                es_l.close()
                with scope(f"ag{l}"):
                    nc.gpsimd.collective_compute(
                        "AllGather", mybir.AluOpType.bypass, replica_groups=rg,
                        ins=[shards[l][:]], outs=[tables[l + 1][:]])

        # ---- graph mean pool (fused inv_cnt one-hot) ----
        es_l = ExitStack(); es_l.enter_context(scope("pool"))
        xt_fin = xt[L % 2]
        pool_ps = zps.tile([128, GPC], F32, tag="z", name="pool_ps")
        for k in range(NB):
            t_ps = tps.tile([128, 128], BF16, tag="tps", name=f"tp_pool{k}")
            nc.tensor.transpose(out=t_ps[:], in_=xt_fin[:, k * 128:(k + 1) * 128],
                                identity=ident[:])
            xs = tbufp.tile([128, D], BF16, tag="tsb", name=f"xs{k}")
            nc.vector.tensor_copy(out=xs[:], in_=t_ps[:])
            Gp = sp.tile([128, GPC], BF16, tag="Gp", name=f"Gp{k}")
            nc.vector.tensor_scalar(out=Gp[:], in0=iotaG[:],
                                    scalar1=bloc_s[:, k:k + 1], scalar2=wpool_s[:, k:k + 1],
                                    op0=mybir.AluOpType.is_equal, op1=mybir.AluOpType.mult)
            nc.tensor.matmul(out=pool_ps[:], lhsT=xs[:], rhs=Gp[:],
                             start=(k == 0), stop=(k == NB - 1))
        pool_sb = headp.tile([128, GPC], BF16, tag="poolsb")
        nc.scalar.activation(out=pool_sb[:], in_=pool_ps[:],
                             func=mybir.ActivationFunctionType.Copy)

        # ---- head ----
        h_ps = zps.tile([HD, GPC], F32, tag="z", name="h_ps")
        nc.tensor.matmul(out=h_ps[:], lhsT=w1_s[:], rhs=pool_sb[:], start=True, stop=True)
        h_sb = headp.tile([HD, GPC], BF16, tag="hsb")
        nc.scalar.activation(out=h_sb[:], in_=h_ps[:],
                             func=mybir.ActivationFunctionType.Relu, bias=b1_s[:, 0:1])
        o_ps = zps.tile([1, GPC], F32, tag="z", name="o_ps")
        nc.tensor.matmul(out=o_ps[:], lhsT=w2_s[:], rhs=h_sb[:], start=True, stop=True)
        o_sb = headp.tile([1, GPC], F32, tag="osb")
        nc.vector.tensor_tensor(out=o_sb[:], in0=o_ps[:],
                                in1=b2_s[:].to_broadcast([1, GPC]), op=mybir.AluOpType.add)
        nc.sync.dma_start(out=out_p.ap()[None, :], in_=o_sb[:])
        es_l.close()

    nc.compile()
    return nc


def kernel(**inputs):
    global LAST_RESULT
    x = np.asarray(inputs["x"], np.float32)
    esrc = np.asarray(inputs["edge_src"], np.int64)
    edst = np.asarray(inputs["edge_dst"], np.int64)
    bids = np.asarray(inputs["batch_ids"], np.int64)
    Wl = np.asarray(inputs["Wl"], np.float32)
    Wr = np.asarray(inputs["Wr"], np.float32)
    gamma = np.asarray(inputs["gamma"], np.float32)
    beta = np.asarray(inputs["beta"], np.float32)
    hW1 = np.asarray(inputs["head_W1"], np.float32)
    hb1 = np.asarray(inputs["head_b1"], np.float32)
    hW2 = np.asarray(inputs["head_W2"], np.float32)
    hb2 = np.asarray(inputs["head_b2"], np.float32)

    p = _preprocess(x, esrc, edst, bids)
    nc = _build(p, Wl, Wr, gamma, beta, hW1, hb1, hW2, hb2)

    gb = np.stack([gamma.T, beta.T], axis=-1).astype(np.float32)  # [D, L, 2]
    shared = {
        "table0": p.table0,
        "wl": Wl.astype(ml_dtypes.bfloat16),
        "wr": Wr.astype(ml_dtypes.bfloat16),
        "gb": gb,
        "w1": hW1.astype(ml_dtypes.bfloat16),
        "b1": hb1.reshape(HD, 1).astype(np.float32),
        "w2": hW2.astype(ml_dtypes.bfloat16),
        "b2": hb2.reshape(1, 1).astype(np.float32),
    }
    in_maps = []
    for c in range(C):
        m = dict(shared)
        m["xt0"] = p.xt0[c]
        m["eidx"] = p.eidx[c]
        m["ed"] = p.ed[c]
        m["invdegB"] = p.invdegB[c]
        m["wpool"] = p.wpool[c]
        m["bloc"] = p.bloc[c]
        m["mtail"] = p.mask_tail[c]
        in_maps.append(m)

    trace = bool(int(os.environ.get("GNN_TRACE", "0")))
    res = run_bass_kernel_spmd(nc, in_maps, core_ids=list(range(C)), trace=trace)
    LAST_RESULT = res
    out = np.concatenate([np.asarray(res.results[c]["out"], np.float32) for c in range(C)])
    return out


# revision 8
# speedup vs baseline: 1.0529x; 1.0529x over previous
"""BaselineGNN (SAGEConv-mean x3 + BN + relu, graph mean-pool, MLP head) on 8 Trainium2 cores.

Strategy (v2):
  - Nodes/edges sharded by graph across 8 cores; each core owns the destination
    nodes (and all in-edges) of 512 consecutive graphs.
  - Node features in a replicated [8*PN, 128] bf16 DRAM table; per-edge source
    rows fetched with dma_gather. Edges are bucketed by (supergroup of 16
    dst-windows, src-window, dst-window) so one gather call covers a whole
    (supergroup, src-window) span (~8k rows) -> few, large SWDGE calls.
  - Aggregation: per 128-edge tile, S = one-hot(dst slot) built on DVE in
    multi-tile is_equal ops; PE accumulates g.T @ S into a [128, 2048] f32
    PSUM supergroup tile (dst windows of 128).
  - Raw sums are scaled by 1/deg via a prebuilt column-broadcast invdeg matrix
    during the PSUM->SBUF copy (one DVE op per supergroup) - no per-tile scaling.
  - x_new_T = Wl.T@agg_T + Wr.T@x_T per 512-node block, interleaved per
    supergroup; BN batch stats via ScalarE accumulators + [128,2] AllReduce;
    scale+shift+relu fused in one ScalarE activation.
  - Updated shard PE-transposed back to [node, dim] rows, AllGathered into the
    next layer's table. Graph mean-pool via fused (is_equal * inv_cnt) one-hot
    matmuls; 2-layer MLP head.
"""
import os
import numpy as np
import ml_dtypes

from concourse import bass, bacc, mybir
from concourse.bass_utils import run_bass_kernel_spmd
from concourse.masks import make_identity
import concourse.tile as tile

BF16 = mybir.dt.bfloat16
F16 = mybir.dt.float16
F32 = mybir.dt.float32
I16 = mybir.dt.int16
I32 = mybir.dt.int32

C = 8            # cores
D = 128          # feature dim
HD = 64          # head hidden dim
L = 3            # layers
WDST = 128       # dst window (one-hot width / PSUM sub-window)
KSG = 16         # dst windows per supergroup (PSUM tile = [128, KSG*128] f32)
BLK = 512        # node block for update matmuls
TS = 16          # matmul tiles per S-build op
MAXCALL = 2016   # max idxs per dma_gather call: n/16+1 descs must fit the 128-slot SWDGE ring
BN_EPS = 1e-5

LAST_RESULT = None


def _ceil(a, b):
    return -(-a // b) * b


class Plan:
    pass


def _preprocess(x, esrc, edst, bids):
    p = Plan()
    N = x.shape[0]
    G = 4096 if N > 5000 else int(bids.max()) + 1
    GPC = G // C
    p.N, p.G, p.GPC = N, G, GPC

    node_start = np.searchsorted(bids, np.arange(0, G + 1, GPC)).astype(np.int64)
    n_c = np.diff(node_start)
    PN = int(_ceil(int(n_c.max()), BLK))
    p.PN = PN
    p.NB = PN // 128
    p.NBLK = PN // BLK
    NW = PN // WDST
    p.NW = NW
    NSG = -(-NW // KSG)
    p.NSG = NSG
    WS = 2 * PN
    assert WS <= 32767, f"src window {WS} exceeds int16"
    NSW = -(-C * PN // WS)
    p.WS, p.NSW = WS, NSW

    own = np.repeat(np.arange(C), n_c)
    local = np.arange(N) - node_start[own]
    row = own * PN + local

    deg = np.bincount(edst, minlength=N).astype(np.float32)
    invdeg = (1.0 / np.maximum(deg, 1.0)).astype(np.float32)

    e_own = own[edst]
    e_dl = local[edst]
    e_sr = row[esrc]
    e_sw = e_sr // WS
    e_sl = (e_sr % WS).astype(np.int16)
    e_w = e_dl // WDST
    e_sg = e_w // KSG
    # bucket: (supergroup | srcwin | dstwin-in-sg)
    NBUCK = NSG * NSW * KSG
    key = e_own * NBUCK + (e_sg * NSW + e_sw) * KSG + (e_w % KSG)
    order = np.lexsort((e_sr, key))

    counts = np.bincount(key, minlength=C * NBUCK).reshape(C, NBUCK)
    maxc = counts.max(axis=0)
    padded = (_ceil(maxc.astype(np.int64), 128)).astype(np.int64)
    # zero out buckets of nonexistent windows in the (ragged) last supergroup
    for sg in range(NSG):
        kw = min(KSG, NW - sg * KSG)
        for s in range(NSW):
            for wl in range(kw, KSG):
                padded[(sg * NSW + s) * KSG + wl] = 0
    # every PSUM bank (4 windows) needs >=1 matmul so it gets zeroed+written
    for wb in range(NW // 4):
        bids_w = [((w // KSG) * NSW + s) * KSG + (w % KSG)
                  for w in range(wb * 4, wb * 4 + 4) for s in range(NSW)]
        if padded[bids_w].sum() == 0:
            padded[bids_w[0]] = 128
    boff = np.concatenate([[0], np.cumsum(padded)])
    EP = int(boff[-1])
    p.EP = EP

    # gather calls: per (sg, srcwin), chopped at MAXCALL (128-aligned)
    p.calls = []          # [sg] -> list of (sw, pos, n)
    p.sg_pos = []         # [sg] -> (start, end) edge-offset range
    p.tiles_sg = []       # [sg] -> list of (col, w_local)
    p.wfirst, p.wlast = [], []
    for sg in range(NSG):
        kw = min(KSG, NW - sg * KSG)
        calls = []
        sg_start = int(boff[(sg * NSW + 0) * KSG])
        sg_end = int(boff[min((sg * NSW + NSW - 1) * KSG + KSG, NBUCK)])
        for s in range(NSW):
            b0 = (sg * NSW + s) * KSG
            pos = int(boff[b0])
            m = int(padded[b0:b0 + KSG].sum())
            if m > 0:
                nchunks = -(-m // MAXCALL)
                csz = _ceil(-(-m // nchunks), 128)
                while m > 0:
                    n = min(csz, m)
                    calls.append((s, pos, n))
                    pos += n
                    m -= n
        p.calls.append(calls)
        p.sg_pos.append((sg_start, sg_end))
        tl = []
        for s in range(NSW):
            for wl in range(kw):
                b = (sg * NSW + s) * KSG + wl
                for t in range(int(padded[b]) // 128):
                    tl.append((int(boff[b]) // 128 + t, wl))
        p.tiles_sg.append(tl)
        wf = {}
        wl_ = {}
        for (col, w) in tl:
            wb = w // 4
            if wb not in wf:
                wf[wb] = col
            wl_[wb] = col
        p.wfirst.append(wf)
        p.wlast.append(wl_)
    p.max_sg_cols = max(e - s for s, e in p.sg_pos)

    # per-core edge arrays in padded layout
    key_sorted = key[order]
    core_edges = np.searchsorted(key_sorted, np.arange(0, C * NBUCK + 1, NBUCK))
    p.eidx, p.ed = [], []
    for c in range(C):
        sel = order[core_edges[c]:core_edges[c + 1]]
        k_loc = key[sel] - c * NBUCK
        bstart = np.searchsorted(k_loc, np.arange(NBUCK))
        r = np.arange(len(sel)) - bstart[k_loc]
        pos = boff[k_loc] + r
        idx_arr = np.zeros(EP, np.int16)
        d_arr = np.full(EP, -1.0, np.float32)
        idx_arr[pos] = e_sl[sel]
        d_arr[pos] = (e_dl[sel] % WDST).astype(np.float32)
        eidx16 = idx_arr.reshape(EP // 16, 16).T.copy()
        p.eidx.append(np.tile(eidx16, (8, 1)))
        p.ed.append(d_arr.reshape(EP // 128, 128).T.astype(ml_dtypes.bfloat16))

    # initial table + per-core node-side arrays
    tbl0 = np.zeros((C * PN, D), ml_dtypes.bfloat16)
    tbl0[row] = x.astype(ml_dtypes.bfloat16)
    p.table0 = tbl0
    p.xt0, p.invdegB = [], []
    p.wpool, p.bloc, p.mask_tail = [], [], []
    cnt = np.bincount(bids, minlength=G).astype(np.float32)
    inv_cnt = (1.0 / np.maximum(cnt, 1.0)).astype(np.float32)
    MT = min(PN, 1024)
    p.MT = MT
    for c in range(C):
        nc_ = int(n_c[c])
        xt = np.zeros((D, PN), ml_dtypes.bfloat16)
        xt[:, :nc_] = x[node_start[c]:node_start[c + 1]].T.astype(ml_dtypes.bfloat16)
        p.xt0.append(xt)
        iv = np.zeros(PN, np.float32)
        iv[:nc_] = invdeg[node_start[c]:node_start[c + 1]]
        p.invdegB.append(np.tile(iv[None, :].astype(ml_dtypes.bfloat16), (128, 1)))
        wp = np.zeros(PN, np.float32)
        bl = np.full(PN, -1.0, np.float32)
        gids = bids[node_start[c]:node_start[c + 1]]
        wp[:nc_] = inv_cnt[gids]
        bl[:nc_] = (gids - c * GPC).astype(np.float32)
        p.wpool.append(wp.reshape(PN // 128, 128).T.copy())
        p.bloc.append(bl.reshape(PN // 128, 128).T.copy())
        mt = np.zeros(MT, ml_dtypes.bfloat16)
        valid_in_tail = nc_ - (PN - MT)
        if valid_in_tail > 0:
            mt[:valid_in_tail] = 1.0
        p.mask_tail.append(np.tile(mt[None, :], (128, 1)))
    return p


def _build(p, Wl, Wr, gamma, beta, hW1, hb1, hW2, hb2):
    PN, NW, NB, NBLK, NSW, WS, EP = p.PN, p.NW, p.NB, p.NBLK, p.NSW, p.WS, p.EP
    NSG, GPC = p.NSG, p.GPC
    nc = bacc.Bacc('TRN2', target_bir_lowering=False, debug=False,
                   num_devices=C, num_swdge_queues=4, dynamic_dma_scratch_size=32768)

    # ---- parameters ----
    table0 = nc.declare_dram_parameter("table0", [C * PN, D], BF16, isOutput=False)
    xt0 = nc.declare_dram_parameter("xt0", [D, PN], BF16, isOutput=False)
    eidx = nc.declare_dram_parameter("eidx", [128, EP // 16], I16, isOutput=False)
    ed = nc.declare_dram_parameter("ed", [128, EP // 128], BF16, isOutput=False)
    ivb_p = nc.declare_dram_parameter("invdegB", [128, PN], BF16, isOutput=False)
    wl_p = nc.declare_dram_parameter("wl", [L, D, D], BF16, isOutput=False)
    wr_p = nc.declare_dram_parameter("wr", [L, D, D], BF16, isOutput=False)
    gb_p = nc.declare_dram_parameter("gb", [D, L, 2], F32, isOutput=False)
    wpool_p = nc.declare_dram_parameter("wpool", [128, NB], F32, isOutput=False)
    bloc_p = nc.declare_dram_parameter("bloc", [128, NB], F32, isOutput=False)
    mtail_p = nc.declare_dram_parameter("mtail", [128, p.MT], BF16, isOutput=False)
    w1_p = nc.declare_dram_parameter("w1", [D, HD], BF16, isOutput=False)
    b1_p = nc.declare_dram_parameter("b1", [HD, 1], F32, isOutput=False)
    w2_p = nc.declare_dram_parameter("w2", [HD, 1], BF16, isOutput=False)
    b2_p = nc.declare_dram_parameter("b2", [1, 1], F32, isOutput=False)
    out_p = nc.declare_dram_parameter("out", [GPC], F32, isOutput=True)

    # ---- internal DRAM ----
    tables = [table0]
    shards = []
    for l in range(1, L):
        tables.append(nc.dram_tensor(f"table{l}", [C * PN, D], BF16, addr_space="Shared"))
        shards.append(nc.dram_tensor(f"shard{l}", [PN, D], BF16))
    bnin = [nc.dram_tensor(f"bnin{l}", [D, 2], F32) for l in range(L)]
    bnout = [nc.dram_tensor(f"bnout{l}", [D, 2], F32, addr_space="Shared") for l in range(L)]
    rg = [list(range(C))]

    from contextlib import ExitStack
    with tile.TileContext(nc) as tc, ExitStack() as es:
        const = es.enter_context(tc.tile_pool(name="const", bufs=1))
        big = es.enter_context(tc.tile_pool(name="big", bufs=1))
        eidxp = es.enter_context(tc.tile_pool(name="eidxp", bufs=3))
        featp = es.enter_context(tc.tile_pool(name="feat", bufs=6))
        sp = es.enter_context(tc.tile_pool(name="sel", bufs=8))
        aggsb = es.enter_context(tc.tile_pool(name="aggsb", bufs=2))
        sqp = es.enter_context(tc.tile_pool(name="sqp", bufs=2))
        headp = es.enter_context(tc.tile_pool(name="headp", bufs=1))
        smallp = es.enter_context(tc.tile_pool(name="small", bufs=4))
        aggps = es.enter_context(tc.tile_pool(name="aggps", bufs=1, space="PSUM"))
        zps = es.enter_context(tc.tile_pool(name="zps", bufs=2, space="PSUM"))
        tps = es.enter_context(tc.tile_pool(name="tps", bufs=2, space="PSUM"))
        tbufp = es.enter_context(tc.tile_pool(name="tbuf", bufs=4))

        # ---- persistent constants ----
        iota_i = const.tile([128, WDST], I32)
        nc.gpsimd.iota(iota_i[:], pattern=[[1, WDST]], base=0, channel_multiplier=0)
        iota128 = const.tile([128, WDST], BF16)
        nc.vector.tensor_copy(out=iota128[:], in_=iota_i[:])
        iotaG_i = const.tile([128, GPC], I32)
        nc.gpsimd.iota(iotaG_i[:], pattern=[[1, GPC]], base=0, channel_multiplier=0)
        iotaG = const.tile([128, GPC], F16)
        nc.vector.tensor_copy(out=iotaG[:], in_=iotaG_i[:])
        ident = const.tile([128, 128], BF16)
        make_identity(nc, ident[:])

        wl_s = const.tile([128, L * D], BF16)
        wr_s = const.tile([128, L * D], BF16)
        for l in range(L):
            nc.sync.dma_start(out=wl_s[:, l * D:(l + 1) * D], in_=wl_p[l])
            nc.sync.dma_start(out=wr_s[:, l * D:(l + 1) * D], in_=wr_p[l])
        gb_s = const.tile([128, L, 2], F32)
        nc.sync.dma_start(out=gb_s[:], in_=gb_p[:])
        w1_s = const.tile([D, HD], BF16)
        nc.sync.dma_start(out=w1_s[:], in_=w1_p[:])
        b1_s = const.tile([HD, 1], F32)
        nc.sync.dma_start(out=b1_s[:], in_=b1_p[:])
        w2_s = const.tile([HD, 1], BF16)
        nc.sync.dma_start(out=w2_s[:], in_=w2_p[:])
        b2_s = const.tile([1, 1], F32)
        nc.sync.dma_start(out=b2_s[:], in_=b2_p[:])
        wpool_s = const.tile([128, NB], F32)
        nc.sync.dma_start(out=wpool_s[:], in_=wpool_p[:])
        bloc_s = const.tile([128, NB], F32)
        nc.sync.dma_start(out=bloc_s[:], in_=bloc_p[:])
        mtail_s = const.tile([128, p.MT], BF16)
        nc.sync.dma_start(out=mtail_s[:], in_=mtail_p[:])
        eps_s = const.tile([128, 1], F32)
        nc.vector.memset(eps_s[:], BN_EPS)

        ed_s = big.tile([128, EP // 128], BF16, tag="ed")
        nc.sync.dma_start(out=ed_s[:], in_=ed[:])
        ivb_s = big.tile([128, PN], BF16, tag="ivb")
        nc.sync.dma_start(out=ivb_s[:], in_=ivb_p[:])

        xt = [big.tile([D, PN], BF16, tag="xt0", name="xt_a"),
              big.tile([D, PN], BF16, tag="xt1", name="xt_b")]
        nc.sync.dma_start(out=xt[0][:], in_=xt0[:])
        sq_scr = sqp.tile([128, BLK], F32, tag="sqscr")

        MAXSGC = _ceil(p.max_sg_cols // 16, 64)

        scope = nc.named_scope
        for l in range(L):
            tbl = tables[l]
            xt_cur = xt[l % 2]
            xt_nxt = xt[(l + 1) % 2]

            es_l = ExitStack(); es_l.enter_context(scope(f"agg{l}"))
            parts = smallp.tile([128, 2, NBLK], F32, tag="parts", name=f"parts{l}")
            qrot = 0
            for sg in range(NSG):
                kw = min(KSG, NW - sg * KSG)
                cols = kw * 128
                sg_start, sg_end = p.sg_pos[sg]
                eidx_sg = eidxp.tile([128, MAXSGC], I16, tag="eidx", name=f"eidx{l}_{sg}")
                nc.sync.dma_start(out=eidx_sg[:, :(sg_end - sg_start) // 16],
                                  in_=eidx.ap()[:, sg_start // 16:sg_end // 16])
                agg_ps = aggps.tile([128, cols], F32, tag="aggps", name=f"aggps{l}_{sg}")
                gbufs = []
                for (sw, pos, n) in p.calls[sg]:
                    g = featp.tile([128, n // 128, D], BF16, tag="g", name=f"g{l}_{sg}_{len(gbufs)}")
                    nc.gpsimd.dma_gather(
                        out_ap=g[:],
                        in_ap=tbl.ap()[sw * WS:(sw + 1) * WS],
                        idxs_ap=eidx_sg[:, (pos - sg_start) // 16:(pos - sg_start + n) // 16],
                        num_idxs=n, num_idxs_reg=n, elem_size=D,
                        single_packet=(n <= 1024),
                        queue_num=qrot % 4,
                    )
                    qrot += 1
                    gbufs.append((pos // 128, n // 128, g))
                tl = p.tiles_sg[sg]
                wf, wla = p.wfirst[sg], p.wlast[sg]
                gi = 0
                for ci in range(0, len(tl), TS):
                    chunk = tl[ci:ci + TS]
                    ts = len(chunk)
                    c0 = chunk[0][0]
                    S = sp.tile([128, ts, WDST], BF16, tag="S", name=f"S{l}_{sg}_{ci}")
                    nc.vector.tensor_tensor(
                        out=S[:],
                        in0=ed_s[:, c0:c0 + ts].unsqueeze(-1).to_broadcast([128, ts, WDST]),
                        in1=iota128[:].unsqueeze(1).to_broadcast([128, ts, WDST]),
                        op=mybir.AluOpType.is_equal)
                    for j, (col, w) in enumerate(chunk):
                        while not (gbufs[gi][0] <= col < gbufs[gi][0] + gbufs[gi][1]):
                            gi += 1
                        g0, _, g = gbufs[gi]
                        wb = w // 4
                        nc.tensor.matmul(out=agg_ps[:, w * WDST:(w + 1) * WDST],
                                         lhsT=g[:, col - g0, :], rhs=S[:, j, :],
                                         start=(col == wf[wb]), stop=(col == wla[wb]))
                agg_sb = aggsb.tile([128, cols], BF16, tag="aggsb", name=f"aggsb{l}_{sg}")
                nc.vector.tensor_tensor(out=agg_sb[:], in0=agg_ps[:],
                                        in1=ivb_s[:, sg * KSG * 128:sg * KSG * 128 + cols],
                                        op=mybir.AluOpType.mult)
                # ---- update matmuls + BN stat accum for this supergroup ----
                nblk_sg = cols // BLK
                for bi in range(nblk_sg):
                    b = (sg * KSG * 128) // BLK + bi
                    off = bi * BLK
                    g0 = b * BLK
                    z_ps = zps.tile([128, BLK], F32, tag="z", name=f"z{l}_{b}")
                    nc.tensor.matmul(out=z_ps[:], lhsT=wl_s[:, l * D:(l + 1) * D],
                                     rhs=agg_sb[:, off:off + BLK], start=True, stop=False)
                    nc.tensor.matmul(out=z_ps[:], lhsT=wr_s[:, l * D:(l + 1) * D],
                                     rhs=xt_cur[:, g0:g0 + BLK], start=False, stop=True)
                    nc.scalar.activation(out=xt_nxt[:, g0:g0 + BLK], in_=z_ps[:],
                                         func=mybir.ActivationFunctionType.Copy,
                                         accum_out=parts[:, 0, b:b + 1])
                    nc.scalar.activation(out=sq_scr[:], in_=z_ps[:],
                                         func=mybir.ActivationFunctionType.Square,
                                         accum_out=parts[:, 1, b:b + 1])

            es_l.close()
            es_l = ExitStack(); es_l.enter_context(scope(f"bnred{l}"))
            st_loc = smallp.tile([128, 2], F32, tag="stloc", name=f"stloc{l}")
            nc.vector.tensor_reduce(out=st_loc[:], in_=parts[:],
                                    axis=mybir.AxisListType.X, op=mybir.AluOpType.add)
            nc.sync.dma_start(out=bnin[l][:], in_=st_loc[:])
            nc.gpsimd.collective_compute(
                "AllReduce", mybir.AluOpType.add, replica_groups=rg,
                ins=[bnin[l][:]], outs=[bnout[l][:]])
            st = smallp.tile([128, 2], F32, tag="st", name=f"st{l}")
            nc.sync.dma_start(out=st[:], in_=bnout[l][:])

            # scale = gamma * rsqrt(var+eps); shift = beta - mean*scale
            stat = smallp.tile([128, 6], F32, tag="stat", name=f"stat{l}")
            inv_n = 1.0 / float(p.N)
            nc.vector.tensor_scalar(out=stat[:, 0:2], in0=st[:, 0:2], scalar1=inv_n,
                                    scalar2=None, op0=mybir.AluOpType.mult)  # mean, E[x^2]
            nc.vector.tensor_tensor(out=stat[:, 2:3], in0=stat[:, 0:1], in1=stat[:, 0:1],
                                    op=mybir.AluOpType.mult)  # mean^2
            nc.vector.tensor_tensor(out=stat[:, 2:3], in0=stat[:, 1:2], in1=stat[:, 2:3],
                                    op=mybir.AluOpType.subtract)  # var
            nc.scalar.activation(out=stat[:, 3:4], in_=stat[:, 2:3],
                                 func=mybir.ActivationFunctionType.Sqrt, bias=eps_s[:, 0:1])
            nc.vector.reciprocal(out=stat[:, 4:5], in_=stat[:, 3:4])  # rsqrt(var+eps)
            nc.vector.tensor_tensor(out=stat[:, 4:5], in0=stat[:, 4:5],
                                    in1=gb_s[:, l, 0:1], op=mybir.AluOpType.mult)  # scale
            nc.vector.tensor_tensor(out=stat[:, 5:6], in0=stat[:, 0:1], in1=stat[:, 4:5],
                                    op=mybir.AluOpType.mult)
            nc.vector.tensor_tensor(out=stat[:, 5:6], in0=gb_s[:, l, 1:2], in1=stat[:, 5:6],
                                    op=mybir.AluOpType.subtract)  # shift

            es_l.close()
            es_l = ExitStack(); es_l.enter_context(scope(f"bnapp{l}"))
            # ---- BN apply + relu (+ tail mask) ----
            for b in range(NBLK):
                sl = slice(b * BLK, (b + 1) * BLK)
                nc.scalar.activation(out=xt_nxt[:, sl], in_=xt_nxt[:, sl],
                                     func=mybir.ActivationFunctionType.Relu,
                                     scale=stat[:, 4:5], bias=stat[:, 5:6])
            mt0 = PN - p.MT
            nc.vector.tensor_tensor(out=xt_nxt[:, mt0:PN], in0=xt_nxt[:, mt0:PN],
                                    in1=mtail_s[:], op=mybir.AluOpType.mult)

            es_l.close()
            # ---- transpose to [node, dim] + AllGather ----
            if l < L - 1:
                es_l = ExitStack(); es_l.enter_context(scope(f"trans{l}"))
                shard_v = shards[l].ap().rearrange("(k p) d -> p k d", p=128)
                for k in range(NB):
                    t_ps = tps.tile([128, 128], BF16, tag="tps", name=f"tp{l}_{k}")
                    nc.tensor.transpose(out=t_ps[:], in_=xt_nxt[:, k * 128:(k + 1) * 128],
                                        identity=ident[:])
                    t_sb = tbufp.tile([128, 128], BF16, tag="tsb", name=f"ts{l}_{k}")
                    nc.vector.tensor_copy(out=t_sb[:], in_=t_ps[:])
                    nc.sync.dma_start(out=shard_v[:, k, :], in_=t_sb[:])
                es_l.close()
                with scope(f"ag{l}"):
                    nc.gpsimd.collective_compute(
                        "AllGather", mybir.AluOpType.bypass, replica_groups=rg,
                        ins=[shards[l][:]], outs=[tables[l + 1][:]])

        # ---- graph mean pool (fused inv_cnt one-hot) ----
        es_l = ExitStack(); es_l.enter_context(scope("pool"))
        xt_fin = xt[L % 2]
        pool_ps = zps.tile([128, GPC], F32, tag="z", name="pool_ps")
        for k in range(NB):
            t_ps = tps.tile([128, 128], BF16, tag="tps", name=f"tp_pool{k}")
            nc.tensor.transpose(out=t_ps[:], in_=xt_fin[:, k * 128:(k + 1) * 128],
                                identity=ident[:])
            xs = tbufp.tile([128, D], BF16, tag="tsb", name=f"xs{k}")
            nc.vector.tensor_copy(out=xs[:], in_=t_ps[:])
            Gp = sp.tile([128, GPC], BF16, tag="Gp", name=f"Gp{k}")
            nc.vector.tensor_scalar(out=Gp[:], in0=iotaG[:],
                                    scalar1=bloc_s[:, k:k + 1], scalar2=wpool_s[:, k:k + 1],
                                    op0=mybir.AluOpType.is_equal, op1=mybir.AluOpType.mult)
            nc.tensor.matmul(out=pool_ps[:], lhsT=xs[:], rhs=Gp[:],
                             start=(k == 0), stop=(k == NB - 1))
        pool_sb = headp.tile([128, GPC], BF16, tag="poolsb")
        nc.scalar.activation(out=pool_sb[:], in_=pool_ps[:],
                             func=mybir.ActivationFunctionType.Copy)

        # ---- head ----
        h_ps = zps.tile([HD, GPC], F32, tag="z", name="h_ps")
        nc.tensor.matmul(out=h_ps[:], lhsT=w1_s[:], rhs=pool_sb[:], start=True, stop=True)
        h_sb = headp.tile([HD, GPC], BF16, tag="hsb")
        nc.scalar.activation(out=h_sb[:], in_=h_ps[:],
                             func=mybir.ActivationFunctionType.Relu, bias=b1_s[:, 0:1])
        o_ps = zps.tile([1, GPC], F32, tag="z", name="o_ps")
        nc.tensor.matmul(out=o_ps[:], lhsT=w2_s[:], rhs=h_sb[:], start=True, stop=True)
        o_sb = headp.tile([1, GPC], F32, tag="osb")
        nc.vector.tensor_tensor(out=o_sb[:], in0=o_ps[:],
                                in1=b2_s[:].to_broadcast([1, GPC]), op=mybir.AluOpType.add)
        nc.sync.dma_start(out=out_p.ap()[None, :], in_=o_sb[:])
        es_l.close()

    nc.compile()
    return nc


def kernel(**inputs):
    global LAST_RESULT
    x = np.asarray(inputs["x"], np.float32)
    esrc = np.asarray(inputs["edge_src"], np.int64)
    edst = np.asarray(inputs["edge_dst"], np.int64)
    bids = np.asarray(inputs["batch_ids"], np.int64)
    Wl = np.asarray(inputs["Wl"], np.float32)
    Wr = np.asarray(inputs["Wr"], np.float32)
    gamma = np.asarray(inputs["gamma"], np.float32)
    beta = np.asarray(inputs["beta"], np.float32)
    hW1 = np.asarray(inputs["head_W1"], np.float32)
    hb1 = np.asarray(inputs["head_b1"], np.float32)
    hW2 = np.asarray(inputs["head_W2"], np.float32)
    hb2 = np.asarray(inputs["head_b2"], np.float32)

    p = _preprocess(x, esrc, edst, bids)
    nc = _build(p, Wl, Wr, gamma, beta, hW1, hb1, hW2, hb2)

    gb = np.stack([gamma.T, beta.T], axis=-1).astype(np.float32)  # [D, L, 2]
    shared = {
        "table0": p.table0,
        "wl": Wl.astype(ml_dtypes.bfloat16),
        "wr": Wr.astype(ml_dtypes.bfloat16),
        "gb": gb,
        "w1": hW1.astype(ml_dtypes.bfloat16),
        "b1": hb1.reshape(HD, 1).astype(np.float32),
        "w2": hW2.astype(ml_dtypes.bfloat16),
        "b2": hb2.reshape(1, 1).astype(np.float32),
    }
    in_maps = []
    for c in range(C):
        m = dict(shared)
        m["xt0"] = p.xt0[c]
        m["eidx"] = p.eidx[c]
        m["ed"] = p.ed[c]
        m["invdegB"] = p.invdegB[c]
        m["wpool"] = p.wpool[c]
        m["bloc"] = p.bloc[c]
        m["mtail"] = p.mask_tail[c]
        in_maps.append(m)

    trace = bool(int(os.environ.get("GNN_TRACE", "0")))
    res = run_bass_kernel_spmd(nc, in_maps, core_ids=list(range(C)), trace=trace)
    LAST_RESULT = res
    out = np.concatenate([np.asarray(res.results[c]["out"], np.float32) for c in range(C)])
    return out


# revision 10
# speedup vs baseline: 1.1277x; 1.0710x over previous
"""BaselineGNN (SAGEConv-mean x3 + BN + relu, graph mean-pool, MLP head) on 8 Trainium2 cores.

Strategy (v2):
  - Nodes/edges sharded by graph across 8 cores; each core owns the destination
    nodes (and all in-edges) of 512 consecutive graphs.
  - Node features in a replicated [8*PN, 128] bf16 DRAM table; per-edge source
    rows fetched with dma_gather. Edges are bucketed by (supergroup of 16
    dst-windows, src-window, dst-window) so one gather call covers a whole
    (supergroup, src-window) span (~8k rows) -> few, large SWDGE calls.
  - Aggregation: per 128-edge tile, S = one-hot(dst slot) built on DVE in
    multi-tile is_equal ops; PE accumulates g.T @ S into a [128, 2048] f32
    PSUM supergroup tile (dst windows of 128).
  - Raw sums are scaled by 1/deg via a prebuilt column-broadcast invdeg matrix
    during the PSUM->SBUF copy (one DVE op per supergroup) - no per-tile scaling.
  - x_new_T = Wl.T@agg_T + Wr.T@x_T per 512-node block, interleaved per
    supergroup; BN batch stats via ScalarE accumulators + [128,2] AllReduce;
    scale+shift+relu fused in one ScalarE activation.
  - Updated shard PE-transposed back to [node, dim] rows, AllGathered into the
    next layer's table. Graph mean-pool via fused (is_equal * inv_cnt) one-hot
    matmuls; 2-layer MLP head.
"""
import os
import numpy as np
import ml_dtypes

from concourse import bass, bacc, mybir
from concourse.bass_utils import run_bass_kernel_spmd
from concourse.masks import make_identity
import concourse.tile as tile

BF16 = mybir.dt.bfloat16
F16 = mybir.dt.float16
F32 = mybir.dt.float32
I16 = mybir.dt.int16
I32 = mybir.dt.int32

C = 8            # cores
D = 128          # feature dim
HD = 64          # head hidden dim
L = 3            # layers
WDST = 128       # dst window (one-hot width / PSUM sub-window)
KSG = 16         # dst windows per supergroup (PSUM tile = [128, KSG*128] f32)
BLK = 512        # node block for update matmuls
TS = 16          # matmul tiles per S-build op
MAXCALL = 2016   # max idxs per dma_gather call: n/16+1 descs must fit the 128-slot SWDGE ring
BN_EPS = 1e-5

LAST_RESULT = None


def _ceil(a, b):
    return -(-a // b) * b


class Plan:
    pass


def _preprocess(x, esrc, edst, bids):
    p = Plan()
    N = x.shape[0]
    G = 4096 if N > 5000 else int(bids.max()) + 1
    GPC = G // C
    p.N, p.G, p.GPC = N, G, GPC

    node_start = np.searchsorted(bids, np.arange(0, G + 1, GPC)).astype(np.int64)
    n_c = np.diff(node_start)
    PN = int(_ceil(int(n_c.max()), BLK))
    p.PN = PN
    p.NB = PN // 128
    p.NBLK = PN // BLK
    NW = PN // WDST
    p.NW = NW
    NSG = -(-NW // KSG)
    p.NSG = NSG
    WS = 2 * PN
    assert WS <= 32767, f"src window {WS} exceeds int16"
    NSW = -(-C * PN // WS)
    p.WS, p.NSW = WS, NSW

    own = np.repeat(np.arange(C), n_c)
    local = np.arange(N) - node_start[own]
    row = own * PN + local

    deg = np.bincount(edst, minlength=N).astype(np.float32)
    invdeg = (1.0 / np.maximum(deg, 1.0)).astype(np.float32)

    e_own = own[edst]
    e_dl = local[edst]
    e_sr = row[esrc]
    e_sw = e_sr // WS
    e_sl = (e_sr % WS).astype(np.int16)
    e_w = e_dl // WDST
    e_sg = e_w // KSG
    # bucket: (supergroup | srcwin | dstwin-in-sg)
    NBUCK = NSG * NSW * KSG
    key = e_own * NBUCK + (e_sg * NSW + e_sw) * KSG + (e_w % KSG)
    order = np.lexsort((e_sr, key))

    counts = np.bincount(key, minlength=C * NBUCK).reshape(C, NBUCK)
    maxc = counts.max(axis=0)
    padded = (_ceil(maxc.astype(np.int64), 128)).astype(np.int64)
    # zero out buckets of nonexistent windows in the (ragged) last supergroup
    for sg in range(NSG):
        kw = min(KSG, NW - sg * KSG)
        for s in range(NSW):
            for wl in range(kw, KSG):
                padded[(sg * NSW + s) * KSG + wl] = 0
    # every PSUM bank (4 windows) needs >=1 matmul so it gets zeroed+written
    for wb in range(NW // 4):
        bids_w = [((w // KSG) * NSW + s) * KSG + (w % KSG)
                  for w in range(wb * 4, wb * 4 + 4) for s in range(NSW)]
        if padded[bids_w].sum() == 0:
            padded[bids_w[0]] = 128
    boff = np.concatenate([[0], np.cumsum(padded)])
    EP = int(boff[-1])
    p.EP = EP

    # gather calls: per (sg, srcwin), chopped at MAXCALL (128-aligned)
    p.calls = []          # [sg] -> list of (sw, pos, n)
    p.sg_pos = []         # [sg] -> (start, end) edge-offset range
    p.tiles_sg = []       # [sg] -> list of (col, w_local)
    p.wfirst, p.wlast = [], []
    for sg in range(NSG):
        kw = min(KSG, NW - sg * KSG)
        calls = []
        sg_start = int(boff[(sg * NSW + 0) * KSG])
        sg_end = int(boff[min((sg * NSW + NSW - 1) * KSG + KSG, NBUCK)])
        for s in range(NSW):
            b0 = (sg * NSW + s) * KSG
            pos = int(boff[b0])
            m = int(padded[b0:b0 + KSG].sum())
            if m > 0:
                nchunks = -(-m // MAXCALL)
                csz = _ceil(-(-m // nchunks), 128)
                while m > 0:
                    n = min(csz, m)
                    calls.append((s, pos, n))
                    pos += n
                    m -= n
        p.calls.append(calls)
        p.sg_pos.append((sg_start, sg_end))
        tl = []
        for s in range(NSW):
            for wl in range(kw):
                b = (sg * NSW + s) * KSG + wl
                for t in range(int(padded[b]) // 128):
                    tl.append((int(boff[b]) // 128 + t, wl))
        p.tiles_sg.append(tl)
        wf = {}
        wl_ = {}
        for (col, w) in tl:
            wb = w // 4
            if wb not in wf:
                wf[wb] = col
            wl_[wb] = col
        p.wfirst.append(wf)
        p.wlast.append(wl_)
    p.max_sg_cols = max(e - s for s, e in p.sg_pos)

    # per-core edge arrays in padded layout
    key_sorted = key[order]
    core_edges = np.searchsorted(key_sorted, np.arange(0, C * NBUCK + 1, NBUCK))
    p.eidx, p.ed = [], []
    for c in range(C):
        sel = order[core_edges[c]:core_edges[c + 1]]
        k_loc = key[sel] - c * NBUCK
        bstart = np.searchsorted(k_loc, np.arange(NBUCK))
        r = np.arange(len(sel)) - bstart[k_loc]
        pos = boff[k_loc] + r
        idx_arr = np.zeros(EP, np.int16)
        d_arr = np.full(EP, -1.0, np.float32)
        idx_arr[pos] = e_sl[sel]
        d_arr[pos] = (e_dl[sel] % WDST).astype(np.float32)
        eidx16 = idx_arr.reshape(EP // 16, 16).T.copy()
        p.eidx.append(np.tile(eidx16, (8, 1)))
        p.ed.append(d_arr.reshape(EP // 128, 128).T.astype(ml_dtypes.bfloat16))

    # initial table + per-core node-side arrays
    tbl0 = np.zeros((C * PN, D), ml_dtypes.bfloat16)
    tbl0[row] = x.astype(ml_dtypes.bfloat16)
    p.table0 = tbl0
    p.xt0, p.invdegB = [], []
    p.wpool, p.bloc, p.mask_tail = [], [], []
    cnt = np.bincount(bids, minlength=G).astype(np.float32)
    inv_cnt = (1.0 / np.maximum(cnt, 1.0)).astype(np.float32)
    MT = min(PN, 1024)
    p.MT = MT
    for c in range(C):
        nc_ = int(n_c[c])
        xt = np.zeros((D, PN), ml_dtypes.bfloat16)
        xt[:, :nc_] = x[node_start[c]:node_start[c + 1]].T.astype(ml_dtypes.bfloat16)
        p.xt0.append(xt)
        iv = np.zeros(PN, np.float32)
        iv[:nc_] = invdeg[node_start[c]:node_start[c + 1]]
        p.invdegB.append(np.tile(iv[None, :].astype(ml_dtypes.bfloat16), (128, 1)))
        wp = np.zeros(PN, np.float32)
        bl = np.full(PN, -1.0, np.float32)
        gids = bids[node_start[c]:node_start[c + 1]]
        wp[:nc_] = inv_cnt[gids]
        bl[:nc_] = (gids - c * GPC).astype(np.float32)
        p.wpool.append(wp.reshape(PN // 128, 128).T.copy())
        p.bloc.append(bl.reshape(PN // 128, 128).T.copy())
        mt = np.zeros(MT, ml_dtypes.bfloat16)
        valid_in_tail = nc_ - (PN - MT)
        if valid_in_tail > 0:
            mt[:valid_in_tail] = 1.0
        p.mask_tail.append(np.tile(mt[None, :], (128, 1)))
    return p


def _build(p, Wl, Wr, gamma, beta, hW1, hb1, hW2, hb2):
    PN, NW, NB, NBLK, NSW, WS, EP = p.PN, p.NW, p.NB, p.NBLK, p.NSW, p.WS, p.EP
    NSG, GPC = p.NSG, p.GPC
    nc = bacc.Bacc('TRN2', target_bir_lowering=False, debug=False,
                   num_devices=C, num_swdge_queues=4, dynamic_dma_scratch_size=32768)

    # ---- parameters ----
    table0 = nc.declare_dram_parameter("table0", [C * PN, D], BF16, isOutput=False)
    xt0 = nc.declare_dram_parameter("xt0", [D, PN], BF16, isOutput=False)
    eidx = nc.declare_dram_parameter("eidx", [128, EP // 16], I16, isOutput=False)
    ed = nc.declare_dram_parameter("ed", [128, EP // 128], BF16, isOutput=False)
    ivb_p = nc.declare_dram_parameter("invdegB", [128, PN], BF16, isOutput=False)
    wl_p = nc.declare_dram_parameter("wl", [L, D, D], BF16, isOutput=False)
    wr_p = nc.declare_dram_parameter("wr", [L, D, D], BF16, isOutput=False)
    gb_p = nc.declare_dram_parameter("gb", [D, L, 2], F32, isOutput=False)
    wpool_p = nc.declare_dram_parameter("wpool", [128, NB], F32, isOutput=False)
    bloc_p = nc.declare_dram_parameter("bloc", [128, NB], F32, isOutput=False)
    mtail_p = nc.declare_dram_parameter("mtail", [128, p.MT], BF16, isOutput=False)
    w1_p = nc.declare_dram_parameter("w1", [D, HD], BF16, isOutput=False)
    b1_p = nc.declare_dram_parameter("b1", [HD, 1], F32, isOutput=False)
    w2_p = nc.declare_dram_parameter("w2", [HD, 1], BF16, isOutput=False)
    b2_p = nc.declare_dram_parameter("b2", [1, 1], F32, isOutput=False)
    out_p = nc.declare_dram_parameter("out", [GPC], F32, isOutput=True)

    # ---- internal DRAM ----
    tables = [table0]
    shards = []
    for l in range(1, L):
        tables.append(nc.dram_tensor(f"table{l}", [C * PN, D], BF16, addr_space="Shared"))
        shards.append(nc.dram_tensor(f"shard{l}", [PN, D], BF16))
    bnin = [nc.dram_tensor(f"bnin{l}", [D, 2], F32) for l in range(L)]
    bnout = [nc.dram_tensor(f"bnout{l}", [D, 2], F32, addr_space="Shared") for l in range(L)]
    rg = [list(range(C))]

    from contextlib import ExitStack
    with tile.TileContext(nc) as tc, ExitStack() as es:
        const = es.enter_context(tc.tile_pool(name="const", bufs=1))
        big = es.enter_context(tc.tile_pool(name="big", bufs=1))
        eidxp = es.enter_context(tc.tile_pool(name="eidxp", bufs=3))
        featp = es.enter_context(tc.tile_pool(name="feat", bufs=8))
        sp = es.enter_context(tc.tile_pool(name="sel", bufs=3))
        aggsb = es.enter_context(tc.tile_pool(name="aggsb", bufs=2))
        sqp = es.enter_context(tc.tile_pool(name="sqp", bufs=2))
        headp = es.enter_context(tc.tile_pool(name="headp", bufs=1))
        smallp = es.enter_context(tc.tile_pool(name="small", bufs=4))
        aggps = es.enter_context(tc.tile_pool(name="aggps", bufs=1, space="PSUM"))
        zps = es.enter_context(tc.tile_pool(name="zps", bufs=2, space="PSUM"))
        tps = es.enter_context(tc.tile_pool(name="tps", bufs=2, space="PSUM"))
        tbufp = es.enter_context(tc.tile_pool(name="tbuf", bufs=4))

        # ---- persistent constants ----
        iota_i = const.tile([128, WDST], I32)
        nc.gpsimd.iota(iota_i[:], pattern=[[1, WDST]], base=0, channel_multiplier=0)
        iota128 = const.tile([128, WDST], BF16)
        nc.vector.tensor_copy(out=iota128[:], in_=iota_i[:])
        iotaG_i = const.tile([128, GPC], I32)
        nc.gpsimd.iota(iotaG_i[:], pattern=[[1, GPC]], base=0, channel_multiplier=0)
        iotaG = const.tile([128, GPC], F16)
        nc.vector.tensor_copy(out=iotaG[:], in_=iotaG_i[:])
        ident = const.tile([128, 128], BF16)
        make_identity(nc, ident[:])

        wl_s = const.tile([128, L * D], BF16)
        wr_s = const.tile([128, L * D], BF16)
        for l in range(L):
            nc.sync.dma_start(out=wl_s[:, l * D:(l + 1) * D], in_=wl_p[l])
            nc.sync.dma_start(out=wr_s[:, l * D:(l + 1) * D], in_=wr_p[l])
        gb_s = const.tile([128, L, 2], F32)
        nc.sync.dma_start(out=gb_s[:], in_=gb_p[:])
        w1_s = const.tile([D, HD], BF16)
        nc.sync.dma_start(out=w1_s[:], in_=w1_p[:])
        b1_s = const.tile([HD, 1], F32)
        nc.sync.dma_start(out=b1_s[:], in_=b1_p[:])
        w2_s = const.tile([HD, 1], BF16)
        nc.sync.dma_start(out=w2_s[:], in_=w2_p[:])
        b2_s = const.tile([1, 1], F32)
        nc.sync.dma_start(out=b2_s[:], in_=b2_p[:])
        wpool_s = const.tile([128, NB], F32)
        nc.sync.dma_start(out=wpool_s[:], in_=wpool_p[:])
        bloc_s = const.tile([128, NB], F32)
        nc.sync.dma_start(out=bloc_s[:], in_=bloc_p[:])
        mtail_s = const.tile([128, p.MT], BF16)
        nc.sync.dma_start(out=mtail_s[:], in_=mtail_p[:])
        eps_s = const.tile([128, 1], F32)
        nc.vector.memset(eps_s[:], BN_EPS)

        ed_s = big.tile([128, EP // 128], BF16, tag="ed")
        nc.sync.dma_start(out=ed_s[:], in_=ed[:])
        ivb_s = big.tile([128, PN], BF16, tag="ivb")
        nc.sync.dma_start(out=ivb_s[:], in_=ivb_p[:])

        xt = [big.tile([D, PN], BF16, tag="xt0", name="xt_a"),
              big.tile([D, PN], BF16, tag="xt1", name="xt_b")]
        nc.sync.dma_start(out=xt[0][:], in_=xt0[:])
        sq_scr = sqp.tile([128, BLK], F32, tag="sqscr")

        MAXSGC = _ceil(p.max_sg_cols // 16, 64)

        scope = nc.named_scope
        for l in range(L):
            tbl = tables[l]
            xt_cur = xt[l % 2]
            xt_nxt = xt[(l + 1) % 2]

            es_l = ExitStack(); es_l.enter_context(scope(f"agg{l}"))
            parts = smallp.tile([128, 2, NBLK], F32, tag="parts", name=f"parts{l}")
            qrot = 0
            for sg in range(NSG):
                kw = min(KSG, NW - sg * KSG)
                cols = kw * 128
                sg_start, sg_end = p.sg_pos[sg]
                eidx_sg = eidxp.tile([128, MAXSGC], I16, tag="eidx", name=f"eidx{l}_{sg}")
                nc.sync.dma_start(out=eidx_sg[:, :(sg_end - sg_start) // 16],
                                  in_=eidx.ap()[:, sg_start // 16:sg_end // 16])
                agg_ps = aggps.tile([128, cols], F32, tag="aggps", name=f"aggps{l}_{sg}")
                gbufs = []
                for (sw, pos, n) in p.calls[sg]:
                    g = featp.tile([128, n // 128, D], BF16, tag="g", name=f"g{l}_{sg}_{len(gbufs)}")
                    nc.gpsimd.dma_gather(
                        out_ap=g[:],
                        in_ap=tbl.ap()[sw * WS:(sw + 1) * WS],
                        idxs_ap=eidx_sg[:, (pos - sg_start) // 16:(pos - sg_start + n) // 16],
                        num_idxs=n, num_idxs_reg=n, elem_size=D,
                        single_packet=(n <= 1024),
                        queue_num=qrot % 4,
                    )
                    qrot += 1
                    gbufs.append((pos // 128, n // 128, g))
                tl = p.tiles_sg[sg]
                wf, wla = p.wfirst[sg], p.wlast[sg]
                gi = 0
                for ci in range(0, len(tl), TS):
                    chunk = tl[ci:ci + TS]
                    ts = len(chunk)
                    c0 = chunk[0][0]
                    S = sp.tile([128, ts, WDST], BF16, tag="S", name=f"S{l}_{sg}_{ci}")
                    nc.vector.tensor_tensor(
                        out=S[:],
                        in0=ed_s[:, c0:c0 + ts].unsqueeze(-1).to_broadcast([128, ts, WDST]),
                        in1=iota128[:].unsqueeze(1).to_broadcast([128, ts, WDST]),
                        op=mybir.AluOpType.is_equal)
                    for j, (col, w) in enumerate(chunk):
                        while not (gbufs[gi][0] <= col < gbufs[gi][0] + gbufs[gi][1]):
                            gi += 1
                        g0, _, g = gbufs[gi]
                        wb = w // 4
                        nc.tensor.matmul(out=agg_ps[:, w * WDST:(w + 1) * WDST],
                                         lhsT=g[:, col - g0, :], rhs=S[:, j, :],
                                         start=(col == wf[wb]), stop=(col == wla[wb]))
                agg_sb = aggsb.tile([128, cols], BF16, tag="aggsb", name=f"aggsb{l}_{sg}")
                nc.vector.tensor_tensor(out=agg_sb[:], in0=agg_ps[:],
                                        in1=ivb_s[:, sg * KSG * 128:sg * KSG * 128 + cols],
                                        op=mybir.AluOpType.mult)
                # ---- update matmuls + BN stat accum for this supergroup ----
                nblk_sg = cols // BLK
                for bi in range(nblk_sg):
                    b = (sg * KSG * 128) // BLK + bi
                    off = bi * BLK
                    g0 = b * BLK
                    z_ps = zps.tile([128, BLK], F32, tag="z", name=f"z{l}_{b}")
                    nc.tensor.matmul(out=z_ps[:], lhsT=wl_s[:, l * D:(l + 1) * D],
                                     rhs=agg_sb[:, off:off + BLK], start=True, stop=False)
                    nc.tensor.matmul(out=z_ps[:], lhsT=wr_s[:, l * D:(l + 1) * D],
                                     rhs=xt_cur[:, g0:g0 + BLK], start=False, stop=True)
                    nc.scalar.activation(out=xt_nxt[:, g0:g0 + BLK], in_=z_ps[:],
                                         func=mybir.ActivationFunctionType.Copy,
                                         accum_out=parts[:, 0, b:b + 1])
                    nc.scalar.activation(out=sq_scr[:], in_=z_ps[:],
                                         func=mybir.ActivationFunctionType.Square,
                                         accum_out=parts[:, 1, b:b + 1])

            es_l.close()
            es_l = ExitStack(); es_l.enter_context(scope(f"bnred{l}"))
            st_loc = smallp.tile([128, 2], F32, tag="stloc", name=f"stloc{l}")
            nc.vector.tensor_reduce(out=st_loc[:], in_=parts[:],
                                    axis=mybir.AxisListType.X, op=mybir.AluOpType.add)
            nc.sync.dma_start(out=bnin[l][:], in_=st_loc[:])
            nc.gpsimd.collective_compute(
                "AllReduce", mybir.AluOpType.add, replica_groups=rg,
                ins=[bnin[l][:]], outs=[bnout[l][:]])
            st = smallp.tile([128, 2], F32, tag="st", name=f"st{l}")
            nc.sync.dma_start(out=st[:], in_=bnout[l][:])

            # scale = gamma * rsqrt(var+eps); shift = beta - mean*scale
            stat = smallp.tile([128, 6], F32, tag="stat", name=f"stat{l}")
            inv_n = 1.0 / float(p.N)
            nc.vector.tensor_scalar(out=stat[:, 0:2], in0=st[:, 0:2], scalar1=inv_n,
                                    scalar2=None, op0=mybir.AluOpType.mult)  # mean, E[x^2]
            nc.vector.tensor_tensor(out=stat[:, 2:3], in0=stat[:, 0:1], in1=stat[:, 0:1],
                                    op=mybir.AluOpType.mult)  # mean^2
            nc.vector.tensor_tensor(out=stat[:, 2:3], in0=stat[:, 1:2], in1=stat[:, 2:3],
                                    op=mybir.AluOpType.subtract)  # var
            nc.scalar.activation(out=stat[:, 3:4], in_=stat[:, 2:3],
                                 func=mybir.ActivationFunctionType.Sqrt, bias=eps_s[:, 0:1])
            nc.vector.reciprocal(out=stat[:, 4:5], in_=stat[:, 3:4])  # rsqrt(var+eps)
            nc.vector.tensor_tensor(out=stat[:, 4:5], in0=stat[:, 4:5],
                                    in1=gb_s[:, l, 0:1], op=mybir.AluOpType.mult)  # scale
            nc.vector.tensor_tensor(out=stat[:, 5:6], in0=stat[:, 0:1], in1=stat[:, 4:5],
                                    op=mybir.AluOpType.mult)
            nc.vector.tensor_tensor(out=stat[:, 5:6], in0=gb_s[:, l, 1:2], in1=stat[:, 5:6],
                                    op=mybir.AluOpType.subtract)  # shift

            es_l.close()
            es_l = ExitStack(); es_l.enter_context(scope(f"bnapp{l}"))
            # ---- BN apply + relu (+ tail mask) ----
            for b in range(NBLK):
                sl = slice(b * BLK, (b + 1) * BLK)
                nc.scalar.activation(out=xt_nxt[:, sl], in_=xt_nxt[:, sl],
                                     func=mybir.ActivationFunctionType.Relu,
                                     scale=stat[:, 4:5], bias=stat[:, 5:6])
            mt0 = PN - p.MT
            nc.vector.tensor_tensor(out=xt_nxt[:, mt0:PN], in0=xt_nxt[:, mt0:PN],
                                    in1=mtail_s[:], op=mybir.AluOpType.mult)

            es_l.close()
            # ---- transpose to [node, dim] + AllGather ----
            if l < L - 1:
                es_l = ExitStack(); es_l.enter_context(scope(f"trans{l}"))
                shard_v = shards[l].ap().rearrange("(k p) d -> p k d", p=128)
                for k in range(NB):
                    t_ps = tps.tile([128, 128], BF16, tag="tps", name=f"tp{l}_{k}")
                    nc.tensor.transpose(out=t_ps[:], in_=xt_nxt[:, k * 128:(k + 1) * 128],
                                        identity=ident[:])
                    t_sb = tbufp.tile([128, 128], BF16, tag="tsb", name=f"ts{l}_{k}")
                    nc.vector.tensor_copy(out=t_sb[:], in_=t_ps[:])
                    nc.sync.dma_start(out=shard_v[:, k, :], in_=t_sb[:])
                es_l.close()
                with scope(f"ag{l}"):
                    nc.gpsimd.collective_compute(
                        "AllGather", mybir.AluOpType.bypass, replica_groups=rg,
                        ins=[shards[l][:]], outs=[tables[l + 1][:]])

        # ---- graph mean pool (fused inv_cnt one-hot) ----
        es_l = ExitStack(); es_l.enter_context(scope("pool"))
        xt_fin = xt[L % 2]
        pool_ps = zps.tile([128, GPC], F32, tag="z", name="pool_ps")
        for k in range(NB):
            t_ps = tps.tile([128, 128], BF16, tag="tps", name=f"tp_pool{k}")
            nc.tensor.transpose(out=t_ps[:], in_=xt_fin[:, k * 128:(k + 1) * 128],
                                identity=ident[:])
            xs = tbufp.tile([128, D], BF16, tag="tsb", name=f"xs{k}")
            nc.vector.tensor_copy(out=xs[:], in_=t_ps[:])
            Gp = sp.tile([128, GPC], BF16, tag="Gp", name=f"Gp{k}")
            nc.vector.tensor_scalar(out=Gp[:], in0=iotaG[:],
                                    scalar1=bloc_s[:, k:k + 1], scalar2=wpool_s[:, k:k + 1],
                                    op0=mybir.AluOpType.is_equal, op1=mybir.AluOpType.mult)
            nc.tensor.matmul(out=pool_ps[:], lhsT=xs[:], rhs=Gp[:],
                             start=(k == 0), stop=(k == NB - 1))
        pool_sb = headp.tile([128, GPC], BF16, tag="poolsb")
        nc.scalar.activation(out=pool_sb[:], in_=pool_ps[:],
                             func=mybir.ActivationFunctionType.Copy)

        # ---- head ----
        h_ps = zps.tile([HD, GPC], F32, tag="z", name="h_ps")
        nc.tensor.matmul(out=h_ps[:], lhsT=w1_s[:], rhs=pool_sb[:], start=True, stop=True)
        h_sb = headp.tile([HD, GPC], BF16, tag="hsb")
        nc.scalar.activation(out=h_sb[:], in_=h_ps[:],
                             func=mybir.ActivationFunctionType.Relu, bias=b1_s[:, 0:1])
        o_ps = zps.tile([1, GPC], F32, tag="z", name="o_ps")
        nc.tensor.matmul(out=o_ps[:], lhsT=w2_s[:], rhs=h_sb[:], start=True, stop=True)
        o_sb = headp.tile([1, GPC], F32, tag="osb")
        nc.vector.tensor_tensor(out=o_sb[:], in0=o_ps[:],
                                in1=b2_s[:].to_broadcast([1, GPC]), op=mybir.AluOpType.add)
        nc.sync.dma_start(out=out_p.ap()[None, :], in_=o_sb[:])
        es_l.close()

    nc.compile()
    return nc


def kernel(**inputs):
    global LAST_RESULT
    x = np.asarray(inputs["x"], np.float32)
    esrc = np.asarray(inputs["edge_src"], np.int64)
    edst = np.asarray(inputs["edge_dst"], np.int64)
    bids = np.asarray(inputs["batch_ids"], np.int64)
    Wl = np.asarray(inputs["Wl"], np.float32)
    Wr = np.asarray(inputs["Wr"], np.float32)
    gamma = np.asarray(inputs["gamma"], np.float32)
    beta = np.asarray(inputs["beta"], np.float32)
    hW1 = np.asarray(inputs["head_W1"], np.float32)
    hb1 = np.asarray(inputs["head_b1"], np.float32)
    hW2 = np.asarray(inputs["head_W2"], np.float32)
    hb2 = np.asarray(inputs["head_b2"], np.float32)

    p = _preprocess(x, esrc, edst, bids)
    nc = _build(p, Wl, Wr, gamma, beta, hW1, hb1, hW2, hb2)

    gb = np.stack([gamma.T, beta.T], axis=-1).astype(np.float32)  # [D, L, 2]
    shared = {
        "table0": p.table0,
        "wl": Wl.astype(ml_dtypes.bfloat16),
        "wr": Wr.astype(ml_dtypes.bfloat16),
        "gb": gb,
        "w1": hW1.astype(ml_dtypes.bfloat16),
        "b1": hb1.reshape(HD, 1).astype(np.float32),
        "w2": hW2.astype(ml_dtypes.bfloat16),
        "b2": hb2.reshape(1, 1).astype(np.float32),
    }
    in_maps = []
    for c in range(C):
        m = dict(shared)
        m["xt0"] = p.xt0[c]
        m["eidx"] = p.eidx[c]
        m["ed"] = p.ed[c]
        m["invdegB"] = p.invdegB[c]
        m["wpool"] = p.wpool[c]
        m["bloc"] = p.bloc[c]
        m["mtail"] = p.mask_tail[c]
        in_maps.append(m)

    trace = bool(int(os.environ.get("GNN_TRACE", "0")))
    res = run_bass_kernel_spmd(nc, in_maps, core_ids=list(range(C)), trace=trace)
    LAST_RESULT = res
    out = np.concatenate([np.asarray(res.results[c]["out"], np.float32) for c in range(C)])
    return out


# revision 13
# speedup vs baseline: 1.2692x; 1.1255x over previous
"""BaselineGNN (SAGEConv-mean x3 + BN + relu, graph mean-pool, MLP head) on 8 Trainium2 cores.

Strategy (v2):
  - Nodes/edges sharded by graph across 8 cores; each core owns the destination
    nodes (and all in-edges) of 512 consecutive graphs.
  - Node features in a replicated [8*PN, 128] bf16 DRAM table; per-edge source
    rows fetched with dma_gather. Edges are bucketed by (supergroup of 16
    dst-windows, src-window, dst-window) so one gather call covers a whole
    (supergroup, src-window) span (~8k rows) -> few, large SWDGE calls.
  - Aggregation: per 128-edge tile, S = one-hot(dst slot) built on DVE in
    multi-tile is_equal ops; PE accumulates g.T @ S into a [128, 2048] f32
    PSUM supergroup tile (dst windows of 128).
  - Raw sums are scaled by 1/deg via a prebuilt column-broadcast invdeg matrix
    during the PSUM->SBUF copy (one DVE op per supergroup) - no per-tile scaling.
  - x_new_T = Wl.T@agg_T + Wr.T@x_T per 512-node block, interleaved per
    supergroup; BN batch stats via ScalarE accumulators + [128,2] AllReduce;
    scale+shift+relu fused in one ScalarE activation.
  - Updated shard PE-transposed back to [node, dim] rows, AllGathered into the
    next layer's table. Graph mean-pool via fused (is_equal * inv_cnt) one-hot
    matmuls; 2-layer MLP head.
"""
import os
import numpy as np
import ml_dtypes

from concourse import bass, bacc, mybir
from concourse.bass_utils import run_bass_kernel_spmd
from concourse.masks import make_identity
import concourse.tile as tile

BF16 = mybir.dt.bfloat16
F16 = mybir.dt.float16
F32 = mybir.dt.float32
I16 = mybir.dt.int16
I32 = mybir.dt.int32

C = 8            # cores
D = 128          # feature dim
HD = 64          # head hidden dim
L = 3            # layers
WDST = 128       # dst window (one-hot width / PSUM sub-window)
KSG = 16         # dst windows per supergroup (PSUM tile = [128, KSG*128] f32)
BLK = 512        # node block for update matmuls
TS = 16          # matmul tiles per S-build op
MAXCALL = 2016   # max idxs per dma_gather call: n/16+1 descs must fit the 128-slot SWDGE ring
BN_EPS = 1e-5

LAST_RESULT = None


def _ceil(a, b):
    return -(-a // b) * b


class Plan:
    pass


def _preprocess(x, esrc, edst, bids):
    p = Plan()
    N = x.shape[0]
    G = 4096 if N > 5000 else int(bids.max()) + 1
    GPC = G // C
    p.N, p.G, p.GPC = N, G, GPC

    node_start = np.searchsorted(bids, np.arange(0, G + 1, GPC)).astype(np.int64)
    n_c = np.diff(node_start)
    PN = int(_ceil(int(n_c.max()), BLK))
    p.PN = PN
    p.NB = PN // 128
    p.NBLK = PN // BLK
    NW = PN // WDST
    p.NW = NW
    NSG = -(-NW // KSG)
    p.NSG = NSG
    WS = 2 * PN
    assert WS <= 32767, f"src window {WS} exceeds int16"
    NSW = -(-C * PN // WS)
    p.WS, p.NSW = WS, NSW

    own = np.repeat(np.arange(C), n_c)
    local = np.arange(N) - node_start[own]
    row = own * PN + local

    deg = np.bincount(edst, minlength=N).astype(np.float32)
    invdeg = (1.0 / np.maximum(deg, 1.0)).astype(np.float32)

    e_own = own[edst]
    e_dl = local[edst]
    e_sr = row[esrc]
    e_sw = e_sr // WS
    e_sl = (e_sr % WS).astype(np.int16)
    e_w = e_dl // WDST
    e_sg = e_w // KSG
    # bucket: (supergroup | srcwin | dstwin-in-sg)
    NBUCK = NSG * NSW * KSG
    key = e_own * NBUCK + (e_sg * NSW + e_sw) * KSG + (e_w % KSG)
    order = np.lexsort((e_sr, key))

    counts = np.bincount(key, minlength=C * NBUCK).reshape(C, NBUCK)
    u = counts.max(axis=0).astype(np.int64)
    # zero nonexistent windows in ragged last sg
    for sg in range(NSG):
        kw = min(KSG, NW - sg * KSG)
        for s in range(NSW):
            for wl in range(kw, KSG):
                u[(sg * NSW + s) * KSG + wl] = 0
    # every PSUM bank needs >=1 position so it gets started
    for wb in range(NW // 4):
        bids_w = [((w // KSG) * NSW + s) * KSG + (w % KSG)
                  for w in range(wb * 4, wb * 4 + 4) for s in range(NSW)]
        if u[bids_w].sum() == 0:
            u[bids_w[0]] = 1

    # packed regions: bump for <=2-window tile spans, end-pad regions to 128
    u_adj = u.copy()
    region_start = {}
    bucket_off = np.zeros(NBUCK, np.int64)
    posG = 0
    for sg in range(NSG):
        kw = min(KSG, NW - sg * KSG)
        for s in range(NSW):
            region_start[(sg, s)] = posG
            o = 0
            for wl in range(kw):
                b = (sg * NSW + s) * KSG + wl
                if o % 128 != 0:
                    need = 128 - (o % 128)
                    if u_adj[b] < need:
                        u_adj[b] = need
                bucket_off[b] = posG + o
                o += int(u_adj[b])
            pad = (-o) % 128
            if pad:
                bl = (sg * NSW + s) * KSG + (kw - 1)
                u_adj[bl] += pad
                o += pad
            posG += o
    EP = int(posG)
    p.EP = EP

    # gather calls per (sg, sw), chopped at MAXCALL (128-aligned)
    p.calls = []
    p.sg_pos = []
    for sg in range(NSG):
        kw = min(KSG, NW - sg * KSG)
        calls = []
        sg_start = region_start[(sg, 0)]
        for s in range(NSW):
            pos = region_start[(sg, s)]
            b0 = (sg * NSW + s) * KSG
            m = int(u_adj[b0:b0 + kw].sum())
            m = _ceil(m, 128)
            if m > 0:
                nchunks = -(-m // MAXCALL)
                csz = _ceil(-(-m // nchunks), 128)
                while m > 0:
                    n = min(csz, m)
                    calls.append((s, pos, n))
                    pos += n
                    m -= n
        p.calls.append(calls)
        last = region_start[(sg, NSW - 1)] + _ceil(
            int(u_adj[(sg * NSW + NSW - 1) * KSG:(sg * NSW + NSW) * KSG].sum()), 128)
        p.sg_pos.append((sg_start, last))
    p.max_sg_cols = max(e - s for s, e in p.sg_pos)

    # uniform position->window map and per-sg one-hot column streams
    wpos = np.full(EP, -1, np.int64)
    for b in range(NBUCK):
        if u_adj[b]:
            wpos[bucket_off[b]:bucket_off[b] + u_adj[b]] = b % KSG
    p.cols_sg = []           # [sg] -> list of (tile_pos, w_local)
    p.bfirst, p.blast = [], []
    for sg in range(NSG):
        kw = min(KSG, NW - sg * KSG)
        cols = []
        for s in range(NSW):
            pos0 = region_start[(sg, s)]
            b0 = (sg * NSW + s) * KSG
            end = pos0 + _ceil(int(u_adj[b0:b0 + kw].sum()), 128)
            for tp in range(pos0 // 128, end // 128):
                ws_in_tile = sorted(set(wpos[tp * 128: tp * 128 + 128].tolist()) - {-1})
                assert 1 <= len(ws_in_tile) <= 2
                for wv in ws_in_tile:
                    cols.append((tp, int(wv)))
        wf, wl_ = {}, {}
        for j, (tp, wv) in enumerate(cols):
            wb = wv // 4
            if wb not in wf:
                wf[wb] = j
            wl_[wb] = j
        p.cols_sg.append(cols)
        p.bfirst.append(wf)
        p.blast.append(wl_)
        for wb in range((kw + 3) // 4):
            assert wb in wf, (sg, wb)
    p.NCOLS = [len(c) for c in p.cols_sg]
    p.TOTCOLS = sum(p.NCOLS)

    # per-core edge arrays in packed layout
    key_sorted = key[order]
    core_edges = np.searchsorted(key_sorted, np.arange(0, C * NBUCK + 1, NBUCK))
    p.eidx, p.ed = [], []
    for c in range(C):
        sel = order[core_edges[c]:core_edges[c + 1]]
        k_loc = key[sel] - c * NBUCK
        bstart = np.searchsorted(k_loc, np.arange(NBUCK))
        r = np.arange(len(sel)) - bstart[k_loc]
        pos = bucket_off[k_loc] + r
        idx_arr = np.zeros(EP, np.int16)
        slot = np.full(EP, -1.0, np.float32)
        idx_arr[pos] = e_sl[sel]
        slot[pos] = (e_dl[sel] % WDST).astype(np.float32)
        eidx16 = idx_arr.reshape(EP // 16, 16).T.copy()
        p.eidx.append(np.tile(eidx16, (8, 1)))
        cols_all = []
        for sg in range(NSG):
            for (tp, wv) in p.cols_sg[sg]:
                seg = slot[tp * 128: tp * 128 + 128]
                wseg = wpos[tp * 128: tp * 128 + 128]
                cols_all.append(np.where(wseg == wv, seg, -1.0))
        p.ed.append(np.stack(cols_all, axis=1).astype(ml_dtypes.bfloat16))

    # initial table + per-core node-side arrays
    tbl0 = np.zeros((C * PN, D), ml_dtypes.bfloat16)
    tbl0[row] = x.astype(ml_dtypes.bfloat16)
    p.table0 = tbl0
    p.xt0, p.invdegB = [], []
    p.wpool, p.bloc, p.mask_tail = [], [], []
    cnt = np.bincount(bids, minlength=G).astype(np.float32)
    inv_cnt = (1.0 / np.maximum(cnt, 1.0)).astype(np.float32)
    MT = min(PN, 1024)
    p.MT = MT
    for c in range(C):
        nc_ = int(n_c[c])
        xt = np.zeros((D, PN), ml_dtypes.bfloat16)
        xt[:, :nc_] = x[node_start[c]:node_start[c + 1]].T.astype(ml_dtypes.bfloat16)
        p.xt0.append(xt)
        iv = np.zeros(PN, np.float32)
        iv[:nc_] = invdeg[node_start[c]:node_start[c + 1]]
        p.invdegB.append(np.tile(iv[None, :].astype(ml_dtypes.bfloat16), (128, 1)))
        wp = np.zeros(PN, np.float32)
        bl = np.full(PN, -1.0, np.float32)
        gids = bids[node_start[c]:node_start[c + 1]]
        wp[:nc_] = inv_cnt[gids]
        bl[:nc_] = (gids - c * GPC).astype(np.float32)
        p.wpool.append(wp.reshape(PN // 128, 128).T.copy())
        p.bloc.append(bl.reshape(PN // 128, 128).T.copy())
        mt = np.zeros(MT, ml_dtypes.bfloat16)
        valid_in_tail = nc_ - (PN - MT)
        if valid_in_tail > 0:
            mt[:valid_in_tail] = 1.0
        p.mask_tail.append(np.tile(mt[None, :], (128, 1)))
    return p


def _build(p, Wl, Wr, gamma, beta, hW1, hb1, hW2, hb2):
    PN, NW, NB, NBLK, NSW, WS, EP = p.PN, p.NW, p.NB, p.NBLK, p.NSW, p.WS, p.EP
    NSG, GPC = p.NSG, p.GPC
    nc = bacc.Bacc('TRN2', target_bir_lowering=False, debug=False,
                   num_devices=C, num_swdge_queues=4, dynamic_dma_scratch_size=32768)

    # ---- parameters ----
    table0 = nc.declare_dram_parameter("table0", [C * PN, D], BF16, isOutput=False)
    xt0 = nc.declare_dram_parameter("xt0", [D, PN], BF16, isOutput=False)
    eidx = nc.declare_dram_parameter("eidx", [128, EP // 16], I16, isOutput=False)
    ed = nc.declare_dram_parameter("ed", [128, p.TOTCOLS], BF16, isOutput=False)
    ivb_p = nc.declare_dram_parameter("invdegB", [128, PN], BF16, isOutput=False)
    wl_p = nc.declare_dram_parameter("wl", [L, D, D], BF16, isOutput=False)
    wr_p = nc.declare_dram_parameter("wr", [L, D, D], BF16, isOutput=False)
    gb_p = nc.declare_dram_parameter("gb", [D, L, 2], F32, isOutput=False)
    wpool_p = nc.declare_dram_parameter("wpool", [128, NB], F32, isOutput=False)
    bloc_p = nc.declare_dram_parameter("bloc", [128, NB], F32, isOutput=False)
    mtail_p = nc.declare_dram_parameter("mtail", [128, p.MT], BF16, isOutput=False)
    w1_p = nc.declare_dram_parameter("w1", [D, HD], BF16, isOutput=False)
    b1_p = nc.declare_dram_parameter("b1", [HD, 1], F32, isOutput=False)
    w2_p = nc.declare_dram_parameter("w2", [HD, 1], BF16, isOutput=False)
    b2_p = nc.declare_dram_parameter("b2", [1, 1], F32, isOutput=False)
    out_p = nc.declare_dram_parameter("out", [GPC], F32, isOutput=True)

    # ---- internal DRAM ----
    tables = [table0]
    tables_sh = [None]
    shards = []
    for l in range(1, L):
        tables.append(nc.dram_tensor(f"tableL{l}", [C * PN, D], BF16))
        tables_sh.append(nc.dram_tensor(f"table{l}", [C * PN, D], BF16, addr_space="Shared"))
        shards.append(nc.dram_tensor(f"shard{l}", [PN, D], BF16))
    bnin = [nc.dram_tensor(f"bnin{l}", [D, 2], F32) for l in range(L)]
    bnout = [nc.dram_tensor(f"bnout{l}", [D, 2], F32, addr_space="Shared") for l in range(L)]
    rg = [list(range(C))]

    from contextlib import ExitStack
    with tile.TileContext(nc) as tc, ExitStack() as es:
        const = es.enter_context(tc.tile_pool(name="const", bufs=1))
        big = es.enter_context(tc.tile_pool(name="big", bufs=1))
        eidxp = es.enter_context(tc.tile_pool(name="eidxp", bufs=3))
        featp = es.enter_context(tc.tile_pool(name="feat", bufs=8))
        sp = es.enter_context(tc.tile_pool(name="sel", bufs=3))
        aggsb = es.enter_context(tc.tile_pool(name="aggsb", bufs=2))
        sqp = es.enter_context(tc.tile_pool(name="sqp", bufs=2))
        headp = es.enter_context(tc.tile_pool(name="headp", bufs=1))
        smallp = es.enter_context(tc.tile_pool(name="small", bufs=4))
        aggps = es.enter_context(tc.tile_pool(name="aggps", bufs=1, space="PSUM"))
        zps = es.enter_context(tc.tile_pool(name="zps", bufs=2, space="PSUM"))
        tps = es.enter_context(tc.tile_pool(name="tps", bufs=2, space="PSUM"))
        tbufp = es.enter_context(tc.tile_pool(name="tbuf", bufs=4))

        # ---- persistent constants ----
        iota_i = const.tile([128, WDST], I32)
        nc.gpsimd.iota(iota_i[:], pattern=[[1, WDST]], base=0, channel_multiplier=0)
        iota128 = const.tile([128, WDST], BF16)
        nc.vector.tensor_copy(out=iota128[:], in_=iota_i[:])
        iotaG_i = const.tile([128, GPC], I32)
        nc.gpsimd.iota(iotaG_i[:], pattern=[[1, GPC]], base=0, channel_multiplier=0)
        iotaG = const.tile([128, GPC], F16)
        nc.vector.tensor_copy(out=iotaG[:], in_=iotaG_i[:])
        ident = const.tile([128, 128], BF16)
        make_identity(nc, ident[:])

        wl_s = const.tile([128, L * D], BF16)
        wr_s = const.tile([128, L * D], BF16)
        for l in range(L):
            nc.sync.dma_start(out=wl_s[:, l * D:(l + 1) * D], in_=wl_p[l])
            nc.sync.dma_start(out=wr_s[:, l * D:(l + 1) * D], in_=wr_p[l])
        gb_s = const.tile([128, L, 2], F32)
        nc.sync.dma_start(out=gb_s[:], in_=gb_p[:])
        w1_s = const.tile([D, HD], BF16)
        nc.sync.dma_start(out=w1_s[:], in_=w1_p[:])
        b1_s = const.tile([HD, 1], F32)
        nc.sync.dma_start(out=b1_s[:], in_=b1_p[:])
        w2_s = const.tile([HD, 1], BF16)
        nc.sync.dma_start(out=w2_s[:], in_=w2_p[:])
        b2_s = const.tile([1, 1], F32)
        nc.sync.dma_start(out=b2_s[:], in_=b2_p[:])
        wpool_s = const.tile([128, NB], F32)
        nc.sync.dma_start(out=wpool_s[:], in_=wpool_p[:])
        bloc_s = const.tile([128, NB], F32)
        nc.sync.dma_start(out=bloc_s[:], in_=bloc_p[:])
        mtail_s = const.tile([128, p.MT], BF16)
        nc.sync.dma_start(out=mtail_s[:], in_=mtail_p[:])
        eps_s = const.tile([128, 1], F32)
        nc.vector.memset(eps_s[:], BN_EPS)

        ed_s = big.tile([128, p.TOTCOLS], BF16, tag="ed")
        nc.sync.dma_start(out=ed_s[:], in_=ed[:])
        ivb_s = big.tile([128, PN], BF16, tag="ivb")
        nc.sync.dma_start(out=ivb_s[:], in_=ivb_p[:])

        xt = [big.tile([D, PN], BF16, tag="xt0", name="xt_a"),
              big.tile([D, PN], BF16, tag="xt1", name="xt_b")]
        nc.sync.dma_start(out=xt[0][:], in_=xt0[:])
        sq_scr = sqp.tile([128, BLK], F32, tag="sqscr")

        MAXSGC = _ceil(p.max_sg_cols // 16, 64)

        scope = nc.named_scope
        for l in range(L):
            tbl = tables[l]
            xt_cur = xt[l % 2]
            xt_nxt = xt[(l + 1) % 2]

            es_l = ExitStack(); es_l.enter_context(scope(f"agg{l}"))
            parts = smallp.tile([128, 2, NBLK], F32, tag="parts", name=f"parts{l}")
            qrot = 0
            col_base = 0
            for sg in range(NSG):
                kw = min(KSG, NW - sg * KSG)
                cols = kw * 128
                sg_start, sg_end = p.sg_pos[sg]
                eidx_sg = eidxp.tile([128, MAXSGC], I16, tag="eidx", name=f"eidx{l}_{sg}")
                nc.sync.dma_start(out=eidx_sg[:, :(sg_end - sg_start) // 16],
                                  in_=eidx.ap()[:, sg_start // 16:sg_end // 16])
                agg_ps = aggps.tile([128, cols], F32, tag="aggps", name=f"aggps{l}_{sg}")
                gbufs = []
                for (sw, pos, n) in p.calls[sg]:
                    g = featp.tile([128, n // 128, D], BF16, tag="g", name=f"g{l}_{sg}_{len(gbufs)}")
                    nc.gpsimd.dma_gather(
                        out_ap=g[:],
                        in_ap=tbl.ap()[sw * WS:(sw + 1) * WS],
                        idxs_ap=eidx_sg[:, (pos - sg_start) // 16:(pos - sg_start + n) // 16],
                        num_idxs=n, num_idxs_reg=n, elem_size=D,
                        single_packet=(n <= 1024),
                        queue_num=qrot % 4,
                    )
                    qrot += 1
                    gbufs.append((pos // 128, n // 128, g))
                colstream = p.cols_sg[sg]
                wf, wla = p.bfirst[sg], p.blast[sg]
                gi = 0
                for ci in range(0, len(colstream), TS):
                    chunk = colstream[ci:ci + TS]
                    ts = len(chunk)
                    c0 = col_base + ci
                    S = sp.tile([128, ts, WDST], BF16, tag="S", name=f"S{l}_{sg}_{ci}")
                    nc.vector.tensor_tensor(
                        out=S[:],
                        in0=ed_s[:, c0:c0 + ts].unsqueeze(-1).to_broadcast([128, ts, WDST]),
                        in1=iota128[:].unsqueeze(1).to_broadcast([128, ts, WDST]),
                        op=mybir.AluOpType.is_equal)
                    for j, (tp, w) in enumerate(chunk):
                        while not (gbufs[gi][0] <= tp < gbufs[gi][0] + gbufs[gi][1]):
                            gi += 1
                        g0, _, g = gbufs[gi]
                        wb = w // 4
                        nc.tensor.matmul(out=agg_ps[:, w * WDST:(w + 1) * WDST],
                                         lhsT=g[:, tp - g0, :], rhs=S[:, j, :],
                                         start=(ci + j == wf[wb]), stop=(ci + j == wla[wb]))
                col_base += len(colstream)
                agg_sb = aggsb.tile([128, cols], BF16, tag="aggsb", name=f"aggsb{l}_{sg}")
                nc.vector.tensor_tensor(out=agg_sb[:], in0=agg_ps[:],
                                        in1=ivb_s[:, sg * KSG * 128:sg * KSG * 128 + cols],
                                        op=mybir.AluOpType.mult)
                # ---- update matmuls + BN stat accum for this supergroup ----
                nblk_sg = cols // BLK
                for bi in range(nblk_sg):
                    b = (sg * KSG * 128) // BLK + bi
                    off = bi * BLK
                    g0 = b * BLK
                    z_ps = zps.tile([128, BLK], F32, tag="z", name=f"z{l}_{b}")
                    nc.tensor.matmul(out=z_ps[:], lhsT=wl_s[:, l * D:(l + 1) * D],
                                     rhs=agg_sb[:, off:off + BLK], start=True, stop=False)
                    nc.tensor.matmul(out=z_ps[:], lhsT=wr_s[:, l * D:(l + 1) * D],
                                     rhs=xt_cur[:, g0:g0 + BLK], start=False, stop=True)
                    nc.scalar.activation(out=xt_nxt[:, g0:g0 + BLK], in_=z_ps[:],
                                         func=mybir.ActivationFunctionType.Copy,
                                         accum_out=parts[:, 0, b:b + 1])
                    nc.scalar.activation(out=sq_scr[:], in_=z_ps[:],
                                         func=mybir.ActivationFunctionType.Square,
                                         accum_out=parts[:, 1, b:b + 1])

            es_l.close()
            es_l = ExitStack(); es_l.enter_context(scope(f"bnred{l}"))
            st_loc = smallp.tile([128, 2], F32, tag="stloc", name=f"stloc{l}")
            nc.vector.tensor_reduce(out=st_loc[:], in_=parts[:],
                                    axis=mybir.AxisListType.X, op=mybir.AluOpType.add)
            nc.sync.dma_start(out=bnin[l][:], in_=st_loc[:])
            nc.gpsimd.collective_compute(
                "AllReduce", mybir.AluOpType.add, replica_groups=rg,
                ins=[bnin[l][:]], outs=[bnout[l][:]])
            st = smallp.tile([128, 2], F32, tag="st", name=f"st{l}")
            nc.sync.dma_start(out=st[:], in_=bnout[l][:])

            # scale = gamma * rsqrt(var+eps); shift = beta - mean*scale
            stat = smallp.tile([128, 6], F32, tag="stat", name=f"stat{l}")
            inv_n = 1.0 / float(p.N)
            nc.vector.tensor_scalar(out=stat[:, 0:2], in0=st[:, 0:2], scalar1=inv_n,
                                    scalar2=None, op0=mybir.AluOpType.mult)  # mean, E[x^2]
            nc.vector.tensor_tensor(out=stat[:, 2:3], in0=stat[:, 0:1], in1=stat[:, 0:1],
                                    op=mybir.AluOpType.mult)  # mean^2
            nc.vector.tensor_tensor(out=stat[:, 2:3], in0=stat[:, 1:2], in1=stat[:, 2:3],
                                    op=mybir.AluOpType.subtract)  # var
            nc.scalar.activation(out=stat[:, 3:4], in_=stat[:, 2:3],
                                 func=mybir.ActivationFunctionType.Sqrt, bias=eps_s[:, 0:1])
            nc.vector.reciprocal(out=stat[:, 4:5], in_=stat[:, 3:4])  # rsqrt(var+eps)
            nc.vector.tensor_tensor(out=stat[:, 4:5], in0=stat[:, 4:5],
                                    in1=gb_s[:, l, 0:1], op=mybir.AluOpType.mult)  # scale
            nc.vector.tensor_tensor(out=stat[:, 5:6], in0=stat[:, 0:1], in1=stat[:, 4:5],
                                    op=mybir.AluOpType.mult)
            nc.vector.tensor_tensor(out=stat[:, 5:6], in0=gb_s[:, l, 1:2], in1=stat[:, 5:6],
                                    op=mybir.AluOpType.subtract)  # shift

            es_l.close()
            es_l = ExitStack(); es_l.enter_context(scope(f"bnapp{l}"))
            # ---- BN apply + relu (+ tail mask) ----
            for b in range(NBLK):
                sl = slice(b * BLK, (b + 1) * BLK)
                nc.scalar.activation(out=xt_nxt[:, sl], in_=xt_nxt[:, sl],
                                     func=mybir.ActivationFunctionType.Relu,
                                     scale=stat[:, 4:5], bias=stat[:, 5:6])
            mt0 = PN - p.MT
            nc.vector.tensor_tensor(out=xt_nxt[:, mt0:PN], in0=xt_nxt[:, mt0:PN],
                                    in1=mtail_s[:], op=mybir.AluOpType.mult)

            es_l.close()
            # ---- transpose to [node, dim] + AllGather ----
            if l < L - 1:
                es_l = ExitStack(); es_l.enter_context(scope(f"trans{l}"))
                shard_v = shards[l].ap().rearrange("(k p) d -> p k d", p=128)
                for k in range(NB):
                    t_ps = tps.tile([128, 128], BF16, tag="tps", name=f"tp{l}_{k}")
                    nc.tensor.transpose(out=t_ps[:], in_=xt_nxt[:, k * 128:(k + 1) * 128],
                                        identity=ident[:])
                    t_sb = tbufp.tile([128, 128], BF16, tag="tsb", name=f"ts{l}_{k}")
                    nc.vector.tensor_copy(out=t_sb[:], in_=t_ps[:])
                    nc.sync.dma_start(out=shard_v[:, k, :], in_=t_sb[:])
                es_l.close()
                with scope(f"ag{l}"):
                    nc.gpsimd.collective_compute(
                        "AllGather", mybir.AluOpType.bypass, replica_groups=rg,
                        ins=[shards[l][:]], outs=[tables_sh[l + 1][:]])
                    nc.sync.dma_start(out=tables[l + 1][:], in_=tables_sh[l + 1][:])

        # ---- graph mean pool (fused inv_cnt one-hot) ----
        es_l = ExitStack(); es_l.enter_context(scope("pool"))
        xt_fin = xt[L % 2]
        pool_ps = zps.tile([128, GPC], F32, tag="z", name="pool_ps")
        for k in range(NB):
            t_ps = tps.tile([128, 128], BF16, tag="tps", name=f"tp_pool{k}")
            nc.tensor.transpose(out=t_ps[:], in_=xt_fin[:, k * 128:(k + 1) * 128],
                                identity=ident[:])
            xs = tbufp.tile([128, D], BF16, tag="tsb", name=f"xs{k}")
            nc.vector.tensor_copy(out=xs[:], in_=t_ps[:])
            Gp = sp.tile([128, GPC], BF16, tag="Gp", name=f"Gp{k}")
            nc.vector.tensor_scalar(out=Gp[:], in0=iotaG[:],
                                    scalar1=bloc_s[:, k:k + 1], scalar2=wpool_s[:, k:k + 1],
                                    op0=mybir.AluOpType.is_equal, op1=mybir.AluOpType.mult)
            nc.tensor.matmul(out=pool_ps[:], lhsT=xs[:], rhs=Gp[:],
                             start=(k == 0), stop=(k == NB - 1))
        pool_sb = headp.tile([128, GPC], BF16, tag="poolsb")
        nc.scalar.activation(out=pool_sb[:], in_=pool_ps[:],
                             func=mybir.ActivationFunctionType.Copy)

        # ---- head ----
        h_ps = zps.tile([HD, GPC], F32, tag="z", name="h_ps")
        nc.tensor.matmul(out=h_ps[:], lhsT=w1_s[:], rhs=pool_sb[:], start=True, stop=True)
        h_sb = headp.tile([HD, GPC], BF16, tag="hsb")
        nc.scalar.activation(out=h_sb[:], in_=h_ps[:],
                             func=mybir.ActivationFunctionType.Relu, bias=b1_s[:, 0:1])
        o_ps = zps.tile([1, GPC], F32, tag="z", name="o_ps")
        nc.tensor.matmul(out=o_ps[:], lhsT=w2_s[:], rhs=h_sb[:], start=True, stop=True)
        o_sb = headp.tile([1, GPC], F32, tag="osb")
        nc.vector.tensor_tensor(out=o_sb[:], in0=o_ps[:],
                                in1=b2_s[:].to_broadcast([1, GPC]), op=mybir.AluOpType.add)
        nc.sync.dma_start(out=out_p.ap()[None, :], in_=o_sb[:])
        es_l.close()

    nc.compile()
    return nc


def kernel(**inputs):
    global LAST_RESULT
    x = np.asarray(inputs["x"], np.float32)
    esrc = np.asarray(inputs["edge_src"], np.int64)
    edst = np.asarray(inputs["edge_dst"], np.int64)
    bids = np.asarray(inputs["batch_ids"], np.int64)
    Wl = np.asarray(inputs["Wl"], np.float32)
    Wr = np.asarray(inputs["Wr"], np.float32)
    gamma = np.asarray(inputs["gamma"], np.float32)
    beta = np.asarray(inputs["beta"], np.float32)
    hW1 = np.asarray(inputs["head_W1"], np.float32)
    hb1 = np.asarray(inputs["head_b1"], np.float32)
    hW2 = np.asarray(inputs["head_W2"], np.float32)
    hb2 = np.asarray(inputs["head_b2"], np.float32)

    p = _preprocess(x, esrc, edst, bids)
    nc = _build(p, Wl, Wr, gamma, beta, hW1, hb1, hW2, hb2)

    gb = np.stack([gamma.T, beta.T], axis=-1).astype(np.float32)  # [D, L, 2]
    shared = {
        "table0": p.table0,
        "wl": Wl.astype(ml_dtypes.bfloat16),
        "wr": Wr.astype(ml_dtypes.bfloat16),
        "gb": gb,
        "w1": hW1.astype(ml_dtypes.bfloat16),
        "b1": hb1.reshape(HD, 1).astype(np.float32),
        "w2": hW2.astype(ml_dtypes.bfloat16),
        "b2": hb2.reshape(1, 1).astype(np.float32),
    }
    in_maps = []
    for c in range(C):
        m = dict(shared)
        m["xt0"] = p.xt0[c]
        m["eidx"] = p.eidx[c]
        m["ed"] = p.ed[c]
        m["invdegB"] = p.invdegB[c]
        m["wpool"] = p.wpool[c]
        m["bloc"] = p.bloc[c]
        m["mtail"] = p.mask_tail[c]
        in_maps.append(m)

    trace = bool(int(os.environ.get("GNN_TRACE", "0")))
    res = run_bass_kernel_spmd(nc, in_maps, core_ids=list(range(C)), trace=trace)
    LAST_RESULT = res
    out = np.concatenate([np.asarray(res.results[c]["out"], np.float32) for c in range(C)])
    return out


# revision 14
# speedup vs baseline: 1.9772x; 1.5579x over previous
"""BaselineGNN (SAGEConv-mean x3 + BN + relu, graph mean-pool, MLP head) on 8 Trainium2 cores.

Strategy (v2):
  - Nodes/edges sharded by graph across 8 cores; each core owns the destination
    nodes (and all in-edges) of 512 consecutive graphs.
  - Node features in a replicated [8*PN, 128] bf16 DRAM table; per-edge source
    rows fetched with dma_gather. Edges are bucketed by (supergroup of 16
    dst-windows, src-window, dst-window) so one gather call covers a whole
    (supergroup, src-window) span (~8k rows) -> few, large SWDGE calls.
  - Aggregation: per 128-edge tile, S = one-hot(dst slot) built on DVE in
    multi-tile is_equal ops; PE accumulates g.T @ S into a [128, 2048] f32
    PSUM supergroup tile (dst windows of 128).
  - Raw sums are scaled by 1/deg via a prebuilt column-broadcast invdeg matrix
    during the PSUM->SBUF copy (one DVE op per supergroup) - no per-tile scaling.
  - x_new_T = Wl.T@agg_T + Wr.T@x_T per 512-node block, interleaved per
    supergroup; BN batch stats via ScalarE accumulators + [128,2] AllReduce;
    scale+shift+relu fused in one ScalarE activation.
  - Updated shard PE-transposed back to [node, dim] rows, AllGathered into the
    next layer's table. Graph mean-pool via fused (is_equal * inv_cnt) one-hot
    matmuls; 2-layer MLP head.
"""
import os
import numpy as np
import ml_dtypes

from concourse import bass, bacc, mybir
from concourse.bass_utils import run_bass_kernel_spmd
from concourse.masks import make_identity
import concourse.tile as tile

BF16 = mybir.dt.bfloat16
F16 = mybir.dt.float16
F32 = mybir.dt.float32
I16 = mybir.dt.int16
I32 = mybir.dt.int32

C = 8            # cores
D = 128          # feature dim
HD = 64          # head hidden dim
L = 3            # layers
WDST = 128       # dst window (one-hot width / PSUM sub-window)
KSG = 16         # dst windows per supergroup (PSUM tile = [128, KSG*128] f32)
BLK = 512        # node block for update matmuls
TS = 16          # matmul tiles per S-build op
MAXCALL = 2016   # max idxs per dma_gather call: n/16+1 descs must fit the 128-slot SWDGE ring
BN_EPS = 1e-5

LAST_RESULT = None


def _ceil(a, b):
    return -(-a // b) * b


class Plan:
    pass


def _preprocess(x, esrc, edst, bids):
    p = Plan()
    N = x.shape[0]
    G = 4096 if N > 5000 else int(bids.max()) + 1
    GPC = G // C
    p.N, p.G, p.GPC = N, G, GPC

    node_start = np.searchsorted(bids, np.arange(0, G + 1, GPC)).astype(np.int64)
    n_c = np.diff(node_start)
    PN = int(_ceil(int(n_c.max()), BLK))
    p.PN = PN
    p.NB = PN // 128
    p.NBLK = PN // BLK
    NW = PN // WDST
    p.NW = NW
    NSG = -(-NW // KSG)
    p.NSG = NSG
    WS = 2 * PN
    assert WS <= 32767, f"src window {WS} exceeds int16"
    NSW = -(-C * PN // WS)
    p.WS, p.NSW = WS, NSW

    own = np.repeat(np.arange(C), n_c)
    local = np.arange(N) - node_start[own]
    row = own * PN + local

    deg = np.bincount(edst, minlength=N).astype(np.float32)
    invdeg = (1.0 / np.maximum(deg, 1.0)).astype(np.float32)

    e_own = own[edst]
    e_dl = local[edst]
    e_sr = row[esrc]
    e_sw = e_sr // WS
    e_sl = (e_sr % WS).astype(np.int16)
    e_w = e_dl // WDST
    e_sg = e_w // KSG
    # bucket: (supergroup | srcwin | dstwin-in-sg)
    NBUCK = NSG * NSW * KSG
    key = e_own * NBUCK + (e_sg * NSW + e_sw) * KSG + (e_w % KSG)
    order = np.lexsort((e_sr, key))

    counts = np.bincount(key, minlength=C * NBUCK).reshape(C, NBUCK)
    u = counts.max(axis=0).astype(np.int64)
    # zero nonexistent windows in ragged last sg
    for sg in range(NSG):
        kw = min(KSG, NW - sg * KSG)
        for s in range(NSW):
            for wl in range(kw, KSG):
                u[(sg * NSW + s) * KSG + wl] = 0
    # every PSUM bank needs >=1 position so it gets started
    for wb in range(NW // 4):
        bids_w = [((w // KSG) * NSW + s) * KSG + (w % KSG)
                  for w in range(wb * 4, wb * 4 + 4) for s in range(NSW)]
        if u[bids_w].sum() == 0:
            u[bids_w[0]] = 1

    # packed regions: bump for <=2-window tile spans, end-pad regions to 128
    u_adj = u.copy()
    region_start = {}
    bucket_off = np.zeros(NBUCK, np.int64)
    posG = 0
    for sg in range(NSG):
        kw = min(KSG, NW - sg * KSG)
        for s in range(NSW):
            region_start[(sg, s)] = posG
            o = 0
            for wl in range(kw):
                b = (sg * NSW + s) * KSG + wl
                if o % 128 != 0:
                    need = 128 - (o % 128)
                    if u_adj[b] < need:
                        u_adj[b] = need
                bucket_off[b] = posG + o
                o += int(u_adj[b])
            pad = (-o) % 128
            if pad:
                bl = (sg * NSW + s) * KSG + (kw - 1)
                u_adj[bl] += pad
                o += pad
            posG += o
    EP = int(posG)
    p.EP = EP

    # gather calls per (sg, sw), chopped at MAXCALL (128-aligned)
    p.calls = []
    p.sg_pos = []
    for sg in range(NSG):
        kw = min(KSG, NW - sg * KSG)
        calls = []
        sg_start = region_start[(sg, 0)]
        for s in range(NSW):
            pos = region_start[(sg, s)]
            b0 = (sg * NSW + s) * KSG
            m = int(u_adj[b0:b0 + kw].sum())
            m = _ceil(m, 128)
            if m > 0:
                nchunks = -(-m // MAXCALL)
                csz = _ceil(-(-m // nchunks), 128)
                while m > 0:
                    n = min(csz, m)
                    calls.append((s, pos, n))
                    pos += n
                    m -= n
        p.calls.append(calls)
        last = region_start[(sg, NSW - 1)] + _ceil(
            int(u_adj[(sg * NSW + NSW - 1) * KSG:(sg * NSW + NSW) * KSG].sum()), 128)
        p.sg_pos.append((sg_start, last))
    p.max_sg_cols = max(e - s for s, e in p.sg_pos)

    # uniform position->window map and per-sg one-hot column streams
    wpos = np.full(EP, -1, np.int64)
    for b in range(NBUCK):
        if u_adj[b]:
            wpos[bucket_off[b]:bucket_off[b] + u_adj[b]] = b % KSG
    p.cols_sg = []           # [sg] -> list of (tile_pos, w_local)
    p.bfirst, p.blast = [], []
    for sg in range(NSG):
        kw = min(KSG, NW - sg * KSG)
        cols = []
        for s in range(NSW):
            pos0 = region_start[(sg, s)]
            b0 = (sg * NSW + s) * KSG
            end = pos0 + _ceil(int(u_adj[b0:b0 + kw].sum()), 128)
            for tp in range(pos0 // 128, end // 128):
                ws_in_tile = sorted(set(wpos[tp * 128: tp * 128 + 128].tolist()) - {-1})
                assert 1 <= len(ws_in_tile) <= 2
                for wv in ws_in_tile:
                    cols.append((tp, int(wv)))
        wf, wl_ = {}, {}
        for j, (tp, wv) in enumerate(cols):
            wb = wv // 4
            if wb not in wf:
                wf[wb] = j
            wl_[wb] = j
        p.cols_sg.append(cols)
        p.bfirst.append(wf)
        p.blast.append(wl_)
        for wb in range((kw + 3) // 4):
            assert wb in wf, (sg, wb)
    p.NCOLS = [len(c) for c in p.cols_sg]
    p.TOTCOLS = sum(p.NCOLS)

    # per-core edge arrays in packed layout
    key_sorted = key[order]
    core_edges = np.searchsorted(key_sorted, np.arange(0, C * NBUCK + 1, NBUCK))
    p.eidx, p.ed = [], []
    for c in range(C):
        sel = order[core_edges[c]:core_edges[c + 1]]
        k_loc = key[sel] - c * NBUCK
        bstart = np.searchsorted(k_loc, np.arange(NBUCK))
        r = np.arange(len(sel)) - bstart[k_loc]
        pos = bucket_off[k_loc] + r
        idx_arr = np.zeros(EP, np.int16)
        slot = np.full(EP, -1.0, np.float32)
        idx_arr[pos] = e_sl[sel]
        slot[pos] = (e_dl[sel] % WDST).astype(np.float32)
        eidx16 = idx_arr.reshape(EP // 16, 16).T.copy()
        p.eidx.append(np.tile(eidx16, (8, 1)))
        cols_all = []
        for sg in range(NSG):
            for (tp, wv) in p.cols_sg[sg]:
                seg = slot[tp * 128: tp * 128 + 128]
                wseg = wpos[tp * 128: tp * 128 + 128]
                cols_all.append(np.where(wseg == wv, seg, -1.0))
        p.ed.append(np.stack(cols_all, axis=1).astype(ml_dtypes.bfloat16))

    # initial table + per-core node-side arrays
    tbl0 = np.zeros((C * PN, D), ml_dtypes.bfloat16)
    tbl0[row] = x.astype(ml_dtypes.bfloat16)
    p.table0 = tbl0
    p.xt0, p.invdegB = [], []
    p.wpool, p.bloc, p.mask_tail = [], [], []
    cnt = np.bincount(bids, minlength=G).astype(np.float32)
    inv_cnt = (1.0 / np.maximum(cnt, 1.0)).astype(np.float32)
    MT = min(PN, 1024)
    p.MT = MT
    for c in range(C):
        nc_ = int(n_c[c])
        xt = np.zeros((D, PN), ml_dtypes.bfloat16)
        xt[:, :nc_] = x[node_start[c]:node_start[c + 1]].T.astype(ml_dtypes.bfloat16)
        p.xt0.append(xt)
        iv = np.zeros(PN, np.float32)
        iv[:nc_] = invdeg[node_start[c]:node_start[c + 1]]
        p.invdegB.append(np.tile(iv[None, :].astype(ml_dtypes.bfloat16), (128, 1)))
        wp = np.zeros(PN, np.float32)
        bl = np.full(PN, -1.0, np.float32)
        gids = bids[node_start[c]:node_start[c + 1]]
        wp[:nc_] = inv_cnt[gids]
        bl[:nc_] = (gids - c * GPC).astype(np.float32)
        p.wpool.append(wp.reshape(PN // 128, 128).T.copy())
        p.bloc.append(bl.reshape(PN // 128, 128).T.copy())
        mt = np.zeros(MT, ml_dtypes.bfloat16)
        valid_in_tail = nc_ - (PN - MT)
        if valid_in_tail > 0:
            mt[:valid_in_tail] = 1.0
        p.mask_tail.append(np.tile(mt[None, :], (128, 1)))
    return p


def _build(p, Wl, Wr, gamma, beta, hW1, hb1, hW2, hb2):
    PN, NW, NB, NBLK, NSW, WS, EP = p.PN, p.NW, p.NB, p.NBLK, p.NSW, p.WS, p.EP
    NSG, GPC = p.NSG, p.GPC
    nc = bacc.Bacc('TRN2', target_bir_lowering=False, debug=False,
                   num_devices=C, num_swdge_queues=4, dynamic_dma_scratch_size=32768)

    # ---- parameters ----
    table0 = nc.declare_dram_parameter("table0", [C * PN, D], BF16, isOutput=False)
    xt0 = nc.declare_dram_parameter("xt0", [D, PN], BF16, isOutput=False)
    eidx = nc.declare_dram_parameter("eidx", [128, EP // 16], I16, isOutput=False)
    ed = nc.declare_dram_parameter("ed", [128, p.TOTCOLS], BF16, isOutput=False)
    ivb_p = nc.declare_dram_parameter("invdegB", [128, PN], BF16, isOutput=False)
    wl_p = nc.declare_dram_parameter("wl", [L, D, D], BF16, isOutput=False)
    wr_p = nc.declare_dram_parameter("wr", [L, D, D], BF16, isOutput=False)
    gb_p = nc.declare_dram_parameter("gb", [D, L, 2], F32, isOutput=False)
    wpool_p = nc.declare_dram_parameter("wpool", [128, NB], F32, isOutput=False)
    bloc_p = nc.declare_dram_parameter("bloc", [128, NB], F32, isOutput=False)
    mtail_p = nc.declare_dram_parameter("mtail", [128, p.MT], BF16, isOutput=False)
    w1_p = nc.declare_dram_parameter("w1", [D, HD], BF16, isOutput=False)
    b1_p = nc.declare_dram_parameter("b1", [HD, 1], F32, isOutput=False)
    w2_p = nc.declare_dram_parameter("w2", [HD, 1], BF16, isOutput=False)
    b2_p = nc.declare_dram_parameter("b2", [1, 1], F32, isOutput=False)
    out_p = nc.declare_dram_parameter("out", [GPC], F32, isOutput=True)

    # ---- internal DRAM ----
    tables = [table0]
    shards = []
    for l in range(1, L):
        tables.append(nc.dram_tensor(f"tableL{l}", [C * PN, D], BF16))
        shards.append(nc.dram_tensor(f"shard{l}", [PN, D], BF16))
    bnin = [nc.dram_tensor(f"bnin{l}", [D, 2], F32) for l in range(L)]
    bnout = [nc.dram_tensor(f"bnout{l}", [D, 2], F32, addr_space="Shared") for l in range(L)]
    rg = [list(range(C))]

    from contextlib import ExitStack
    with tile.TileContext(nc) as tc, ExitStack() as es:
        const = es.enter_context(tc.tile_pool(name="const", bufs=1))
        big = es.enter_context(tc.tile_pool(name="big", bufs=1))
        eidxp = es.enter_context(tc.tile_pool(name="eidxp", bufs=3))
        featp = es.enter_context(tc.tile_pool(name="feat", bufs=8))
        sp = es.enter_context(tc.tile_pool(name="sel", bufs=3))
        aggsb = es.enter_context(tc.tile_pool(name="aggsb", bufs=2))
        sqp = es.enter_context(tc.tile_pool(name="sqp", bufs=2))
        headp = es.enter_context(tc.tile_pool(name="headp", bufs=1))
        smallp = es.enter_context(tc.tile_pool(name="small", bufs=4))
        aggps = es.enter_context(tc.tile_pool(name="aggps", bufs=1, space="PSUM"))
        zps = es.enter_context(tc.tile_pool(name="zps", bufs=2, space="PSUM"))
        tps = es.enter_context(tc.tile_pool(name="tps", bufs=2, space="PSUM"))
        tbufp = es.enter_context(tc.tile_pool(name="tbuf", bufs=4))

        # ---- persistent constants ----
        iota_i = const.tile([128, WDST], I32)
        nc.gpsimd.iota(iota_i[:], pattern=[[1, WDST]], base=0, channel_multiplier=0)
        iota128 = const.tile([128, WDST], BF16)
        nc.vector.tensor_copy(out=iota128[:], in_=iota_i[:])
        iotaG_i = const.tile([128, GPC], I32)
        nc.gpsimd.iota(iotaG_i[:], pattern=[[1, GPC]], base=0, channel_multiplier=0)
        iotaG = const.tile([128, GPC], F16)
        nc.vector.tensor_copy(out=iotaG[:], in_=iotaG_i[:])
        ident = const.tile([128, 128], BF16)
        make_identity(nc, ident[:])

        wl_s = const.tile([128, L * D], BF16)
        wr_s = const.tile([128, L * D], BF16)
        for l in range(L):
            nc.sync.dma_start(out=wl_s[:, l * D:(l + 1) * D], in_=wl_p[l])
            nc.sync.dma_start(out=wr_s[:, l * D:(l + 1) * D], in_=wr_p[l])
        gb_s = const.tile([128, L, 2], F32)
        nc.sync.dma_start(out=gb_s[:], in_=gb_p[:])
        w1_s = const.tile([D, HD], BF16)
        nc.sync.dma_start(out=w1_s[:], in_=w1_p[:])
        b1_s = const.tile([HD, 1], F32)
        nc.sync.dma_start(out=b1_s[:], in_=b1_p[:])
        w2_s = const.tile([HD, 1], BF16)
        nc.sync.dma_start(out=w2_s[:], in_=w2_p[:])
        b2_s = const.tile([1, 1], F32)
        nc.sync.dma_start(out=b2_s[:], in_=b2_p[:])
        wpool_s = const.tile([128, NB], F32)
        nc.sync.dma_start(out=wpool_s[:], in_=wpool_p[:])
        bloc_s = const.tile([128, NB], F32)
        nc.sync.dma_start(out=bloc_s[:], in_=bloc_p[:])
        mtail_s = const.tile([128, p.MT], BF16)
        nc.sync.dma_start(out=mtail_s[:], in_=mtail_p[:])
        eps_s = const.tile([128, 1], F32)
        nc.vector.memset(eps_s[:], BN_EPS)

        ed_s = big.tile([128, p.TOTCOLS], BF16, tag="ed")
        nc.sync.dma_start(out=ed_s[:], in_=ed[:])
        ivb_s = big.tile([128, PN], BF16, tag="ivb")
        nc.sync.dma_start(out=ivb_s[:], in_=ivb_p[:])

        xt = [big.tile([D, PN], BF16, tag="xt0", name="xt_a"),
              big.tile([D, PN], BF16, tag="xt1", name="xt_b")]
        nc.sync.dma_start(out=xt[0][:], in_=xt0[:])
        sq_scr = sqp.tile([128, BLK], F32, tag="sqscr")

        MAXSGC = _ceil(p.max_sg_cols // 16, 64)

        scope = nc.named_scope
        for l in range(L):
            tbl = tables[l]
            xt_cur = xt[l % 2]
            xt_nxt = xt[(l + 1) % 2]

            es_l = ExitStack(); es_l.enter_context(scope(f"agg{l}"))
            parts = smallp.tile([128, 2, NBLK], F32, tag="parts", name=f"parts{l}")
            qrot = 0
            col_base = 0
            for sg in range(NSG):
                kw = min(KSG, NW - sg * KSG)
                cols = kw * 128
                sg_start, sg_end = p.sg_pos[sg]
                eidx_sg = eidxp.tile([128, MAXSGC], I16, tag="eidx", name=f"eidx{l}_{sg}")
                nc.sync.dma_start(out=eidx_sg[:, :(sg_end - sg_start) // 16],
                                  in_=eidx.ap()[:, sg_start // 16:sg_end // 16])
                agg_ps = aggps.tile([128, cols], F32, tag="aggps", name=f"aggps{l}_{sg}")
                gbufs = []
                for (sw, pos, n) in p.calls[sg]:
                    g = featp.tile([128, n // 128, D], BF16, tag="g", name=f"g{l}_{sg}_{len(gbufs)}")
                    nc.gpsimd.dma_gather(
                        out_ap=g[:],
                        in_ap=tbl.ap()[sw * WS:(sw + 1) * WS],
                        idxs_ap=eidx_sg[:, (pos - sg_start) // 16:(pos - sg_start + n) // 16],
                        num_idxs=n, num_idxs_reg=n, elem_size=D,
                        single_packet=(n <= 1024),
                        queue_num=qrot % 4,
                    )
                    qrot += 1
                    gbufs.append((pos // 128, n // 128, g))
                colstream = p.cols_sg[sg]
                wf, wla = p.bfirst[sg], p.blast[sg]
                gi = 0
                for ci in range(0, len(colstream), TS):
                    chunk = colstream[ci:ci + TS]
                    ts = len(chunk)
                    c0 = col_base + ci
                    S = sp.tile([128, ts, WDST], BF16, tag="S", name=f"S{l}_{sg}_{ci}")
                    nc.vector.tensor_tensor(
                        out=S[:],
                        in0=ed_s[:, c0:c0 + ts].unsqueeze(-1).to_broadcast([128, ts, WDST]),
                        in1=iota128[:].unsqueeze(1).to_broadcast([128, ts, WDST]),
                        op=mybir.AluOpType.is_equal)
                    for j, (tp, w) in enumerate(chunk):
                        while not (gbufs[gi][0] <= tp < gbufs[gi][0] + gbufs[gi][1]):
                            gi += 1
                        g0, _, g = gbufs[gi]
                        wb = w // 4
                        nc.tensor.matmul(out=agg_ps[:, w * WDST:(w + 1) * WDST],
                                         lhsT=g[:, tp - g0, :], rhs=S[:, j, :],
                                         start=(ci + j == wf[wb]), stop=(ci + j == wla[wb]))
                col_base += len(colstream)
                agg_sb = aggsb.tile([128, cols], BF16, tag="aggsb", name=f"aggsb{l}_{sg}")
                nc.vector.tensor_tensor(out=agg_sb[:], in0=agg_ps[:],
                                        in1=ivb_s[:, sg * KSG * 128:sg * KSG * 128 + cols],
                                        op=mybir.AluOpType.mult)
                # ---- update matmuls + BN stat accum for this supergroup ----
                nblk_sg = cols // BLK
                for bi in range(nblk_sg):
                    b = (sg * KSG * 128) // BLK + bi
                    off = bi * BLK
                    g0 = b * BLK
                    z_ps = zps.tile([128, BLK], F32, tag="z", name=f"z{l}_{b}")
                    nc.tensor.matmul(out=z_ps[:], lhsT=wl_s[:, l * D:(l + 1) * D],
                                     rhs=agg_sb[:, off:off + BLK], start=True, stop=False)
                    nc.tensor.matmul(out=z_ps[:], lhsT=wr_s[:, l * D:(l + 1) * D],
                                     rhs=xt_cur[:, g0:g0 + BLK], start=False, stop=True)
                    nc.scalar.activation(out=xt_nxt[:, g0:g0 + BLK], in_=z_ps[:],
                                         func=mybir.ActivationFunctionType.Copy,
                                         accum_out=parts[:, 0, b:b + 1])
                    nc.scalar.activation(out=sq_scr[:], in_=z_ps[:],
                                         func=mybir.ActivationFunctionType.Square,
                                         accum_out=parts[:, 1, b:b + 1])

            es_l.close()
            es_l = ExitStack(); es_l.enter_context(scope(f"bnred{l}"))
            st_loc = smallp.tile([128, 2], F32, tag="stloc", name=f"stloc{l}")
            nc.vector.tensor_reduce(out=st_loc[:], in_=parts[:],
                                    axis=mybir.AxisListType.X, op=mybir.AluOpType.add)
            nc.sync.dma_start(out=bnin[l][:], in_=st_loc[:])
            nc.gpsimd.collective_compute(
                "AllReduce", mybir.AluOpType.add, replica_groups=rg,
                ins=[bnin[l][:]], outs=[bnout[l][:]])
            st = smallp.tile([128, 2], F32, tag="st", name=f"st{l}")
            nc.sync.dma_start(out=st[:], in_=bnout[l][:])

            # scale = gamma * rsqrt(var+eps); shift = beta - mean*scale
            stat = smallp.tile([128, 6], F32, tag="stat", name=f"stat{l}")
            inv_n = 1.0 / float(p.N)
            nc.vector.tensor_scalar(out=stat[:, 0:2], in0=st[:, 0:2], scalar1=inv_n,
                                    scalar2=None, op0=mybir.AluOpType.mult)  # mean, E[x^2]
            nc.vector.tensor_tensor(out=stat[:, 2:3], in0=stat[:, 0:1], in1=stat[:, 0:1],
                                    op=mybir.AluOpType.mult)  # mean^2
            nc.vector.tensor_tensor(out=stat[:, 2:3], in0=stat[:, 1:2], in1=stat[:, 2:3],
                                    op=mybir.AluOpType.subtract)  # var
            nc.scalar.activation(out=stat[:, 3:4], in_=stat[:, 2:3],
                                 func=mybir.ActivationFunctionType.Sqrt, bias=eps_s[:, 0:1])
            nc.vector.reciprocal(out=stat[:, 4:5], in_=stat[:, 3:4])  # rsqrt(var+eps)
            nc.vector.tensor_tensor(out=stat[:, 4:5], in0=stat[:, 4:5],
                                    in1=gb_s[:, l, 0:1], op=mybir.AluOpType.mult)  # scale
            nc.vector.tensor_tensor(out=stat[:, 5:6], in0=stat[:, 0:1], in1=stat[:, 4:5],
                                    op=mybir.AluOpType.mult)
            nc.vector.tensor_tensor(out=stat[:, 5:6], in0=gb_s[:, l, 1:2], in1=stat[:, 5:6],
                                    op=mybir.AluOpType.subtract)  # shift

            es_l.close()
            es_l = ExitStack(); es_l.enter_context(scope(f"bnapp{l}"))
            # ---- BN apply + relu (+ tail mask) ----
            for b in range(NBLK):
                sl = slice(b * BLK, (b + 1) * BLK)
                nc.scalar.activation(out=xt_nxt[:, sl], in_=xt_nxt[:, sl],
                                     func=mybir.ActivationFunctionType.Relu,
                                     scale=stat[:, 4:5], bias=stat[:, 5:6])
            mt0 = PN - p.MT
            nc.vector.tensor_tensor(out=xt_nxt[:, mt0:PN], in0=xt_nxt[:, mt0:PN],
                                    in1=mtail_s[:], op=mybir.AluOpType.mult)

            es_l.close()
            # ---- transpose to [node, dim] + AllGather ----
            if l < L - 1:
                es_l = ExitStack(); es_l.enter_context(scope(f"trans{l}"))
                shard_v = shards[l].ap().rearrange("(k p) d -> p k d", p=128)
                for k in range(NB):
                    t_ps = tps.tile([128, 128], BF16, tag="tps", name=f"tp{l}_{k}")
                    nc.tensor.transpose(out=t_ps[:], in_=xt_nxt[:, k * 128:(k + 1) * 128],
                                        identity=ident[:])
                    t_sb = tbufp.tile([128, 128], BF16, tag="tsb", name=f"ts{l}_{k}")
                    nc.vector.tensor_copy(out=t_sb[:], in_=t_ps[:])
                    nc.sync.dma_start(out=shard_v[:, k, :], in_=t_sb[:])
                es_l.close()
                with scope(f"ag{l}"):
                    nc.gpsimd.collective_compute(
                        "AllGather", mybir.AluOpType.bypass, replica_groups=rg,
                        ins=[shards[l][:]], outs=[tables[l + 1][:]])

        # ---- graph mean pool (fused inv_cnt one-hot) ----
        es_l = ExitStack(); es_l.enter_context(scope("pool"))
        xt_fin = xt[L % 2]
        pool_ps = zps.tile([128, GPC], F32, tag="z", name="pool_ps")
        for k in range(NB):
            t_ps = tps.tile([128, 128], BF16, tag="tps", name=f"tp_pool{k}")
            nc.tensor.transpose(out=t_ps[:], in_=xt_fin[:, k * 128:(k + 1) * 128],
                                identity=ident[:])
            xs = tbufp.tile([128, D], BF16, tag="tsb", name=f"xs{k}")
            nc.vector.tensor_copy(out=xs[:], in_=t_ps[:])
            Gp = sp.tile([128, GPC], BF16, tag="Gp", name=f"Gp{k}")
            nc.vector.tensor_scalar(out=Gp[:], in0=iotaG[:],
                                    scalar1=bloc_s[:, k:k + 1], scalar2=wpool_s[:, k:k + 1],
                                    op0=mybir.AluOpType.is_equal, op1=mybir.AluOpType.mult)
            nc.tensor.matmul(out=pool_ps[:], lhsT=xs[:], rhs=Gp[:],
                             start=(k == 0), stop=(k == NB - 1))
        pool_sb = headp.tile([128, GPC], BF16, tag="poolsb")
        nc.scalar.activation(out=pool_sb[:], in_=pool_ps[:],
                             func=mybir.ActivationFunctionType.Copy)

        # ---- head ----
        h_ps = zps.tile([HD, GPC], F32, tag="z", name="h_ps")
        nc.tensor.matmul(out=h_ps[:], lhsT=w1_s[:], rhs=pool_sb[:], start=True, stop=True)
        h_sb = headp.tile([HD, GPC], BF16, tag="hsb")
        nc.scalar.activation(out=h_sb[:], in_=h_ps[:],
                             func=mybir.ActivationFunctionType.Relu, bias=b1_s[:, 0:1])
        o_ps = zps.tile([1, GPC], F32, tag="z", name="o_ps")
        nc.tensor.matmul(out=o_ps[:], lhsT=w2_s[:], rhs=h_sb[:], start=True, stop=True)
        o_sb = headp.tile([1, GPC], F32, tag="osb")
        nc.vector.tensor_tensor(out=o_sb[:], in0=o_ps[:],
                                in1=b2_s[:].to_broadcast([1, GPC]), op=mybir.AluOpType.add)
        nc.sync.dma_start(out=out_p.ap()[None, :], in_=o_sb[:])
        es_l.close()

    nc.compile()
    return nc


def kernel(**inputs):
    global LAST_RESULT
    x = np.asarray(inputs["x"], np.float32)
    esrc = np.asarray(inputs["edge_src"], np.int64)
    edst = np.asarray(inputs["edge_dst"], np.int64)
    bids = np.asarray(inputs["batch_ids"], np.int64)
    Wl = np.asarray(inputs["Wl"], np.float32)
    Wr = np.asarray(inputs["Wr"], np.float32)
    gamma = np.asarray(inputs["gamma"], np.float32)
    beta = np.asarray(inputs["beta"], np.float32)
    hW1 = np.asarray(inputs["head_W1"], np.float32)
    hb1 = np.asarray(inputs["head_b1"], np.float32)
    hW2 = np.asarray(inputs["head_W2"], np.float32)
    hb2 = np.asarray(inputs["head_b2"], np.float32)

    p = _preprocess(x, esrc, edst, bids)
    nc = _build(p, Wl, Wr, gamma, beta, hW1, hb1, hW2, hb2)

    gb = np.stack([gamma.T, beta.T], axis=-1).astype(np.float32)  # [D, L, 2]
    shared = {
        "table0": p.table0,
        "wl": Wl.astype(ml_dtypes.bfloat16),
        "wr": Wr.astype(ml_dtypes.bfloat16),
        "gb": gb,
        "w1": hW1.astype(ml_dtypes.bfloat16),
        "b1": hb1.reshape(HD, 1).astype(np.float32),
        "w2": hW2.astype(ml_dtypes.bfloat16),
        "b2": hb2.reshape(1, 1).astype(np.float32),
    }
    in_maps = []
    for c in range(C):
        m = dict(shared)
        m["xt0"] = p.xt0[c]
        m["eidx"] = p.eidx[c]
        m["ed"] = p.ed[c]
        m["invdegB"] = p.invdegB[c]
        m["wpool"] = p.wpool[c]
        m["bloc"] = p.bloc[c]
        m["mtail"] = p.mask_tail[c]
        in_maps.append(m)

    trace = bool(int(os.environ.get("GNN_TRACE", "0")))
    res = run_bass_kernel_spmd(nc, in_maps, core_ids=list(range(C)), trace=trace)
    LAST_RESULT = res
    out = np.concatenate([np.asarray(res.results[c]["out"], np.float32) for c in range(C)])
    return out
